# revision 22
# baseline (speedup 1.0000x reference)
"""Multi-head causal attention (B=4, T=2048, C=768, H=12, HS=64) on 8 trn2 cores.

Sharding: 48 (batch, head) units -> 6 per core. Core c: batch c//2, heads
6*(c%2) .. 6*(c%2)+6. Each core computes a partial output projection
y_partial[T, C] = sum over its 6 heads; host sums the two partials per batch
and adds the bias.

Per-core design (v2 — software-pipelined):
  All inputs are packed on the host in their final SBUF layout (leading 128
  partition dim) so each lands with ONE dma descriptor; x is loaded in four
  512-column chunks so compute starts after ~2 descriptors.

  Q/K weights are pair-stacked on the host (head 2p cols 0:64, head 2p+1
  cols 64:128), so the QK projection psum IS the pair-stacked pairQ/pairK
  layout and a single DVE copy per (p, qk, chunk) replaces the staging
  SBUF->SBUF DMAs.

  scores  ST[tk, tq] = matmul(lhsT=pairK 64-rows, rhs=pairQ 64-rows) with
          tile_position=(64e, 0): both heads of a pair run concurrently in
          disjoint PE row groups.
  softmax no max-subtraction (scores are O(+-8)); exp on ACT (bf16 out);
          row sums ride along as a ones-column appended to V (vaug col 64).
  P@V     OTu[65, tq] accumulated over tk tiles; row 64 = denominator.
  norm    reciprocal_approx_fast on the [1, 2, 512] denominator rows,
          bf16 K=1 broadcast matmul (fast; the fp32 one costs 1.1us/MM),
          one DVE multiply, DMA into pair-stacked otn.
  proj    y[tq, :] = sum_g matmul(lhsT=otn[:, g, tq], rhs=wpt[g]).

  Program order interleaves the 12 (m-chunk, pair) attention units with the
  projection chunks so the PE never idles >3us (HAM stays at K=8/8) and the
  ACT engine (exp: the co-bottleneck at ~120us) is fed continuously.
"""

import numpy as np
import ml_dtypes

import concourse.bacc as bacc
import concourse.bass as bass
import concourse.tile as tile
from concourse import mybir
from concourse import bass_utils

B, T, C = 4, 2048, 768
H, HS = 12, 64
HL = 6            # heads per core
NCT = C // 128    # 6 contraction tiles
NTT = T // 128    # 16 t tiles
NTC = T // 512    # 4 t chunks
SCALE = 1.0 / 8.0  # 1/sqrt(HS)

F32 = mybir.dt.float32
BF16 = mybir.dt.bfloat16


def build_kernel(nc):
    xh = nc.dram_tensor("xh", [128, NCT, T], BF16, kind="ExternalInput").ap()
    wqkp = nc.dram_tensor("wqkp", [128, 3, 2, NCT, 128], BF16,
                          kind="ExternalInput").ap()
    wvh = nc.dram_tensor("wvh", [128, NCT, HL * HS], BF16,
                         kind="ExternalInput").ap()
    wpth = nc.dram_tensor("wpth", [128, 3, C], BF16, kind="ExternalInput").ap()
    y = nc.dram_tensor("y", [T, C], F32, kind="ExternalOutput").ap()

    with tile.TileContext(nc) as tc:
        with (
            tc.tile_pool(name="consts", bufs=1) as consts,
            tc.tile_pool(name="xw", bufs=1) as xw,
            tc.tile_pool(name="pt", bufs=36) as ptp,
            tc.tile_pool(name="small", bufs=2) as small,
            tc.tile_pool(name="ysb", bufs=2) as ysbp,
            # PSUM: st 2x2 banks + otu 1x2 banks + tt 2x1 banks = 8 banks
            tc.tile_pool(name="ps_st", bufs=2, space="PSUM") as ps_st,
            tc.tile_pool(name="ps_otu", bufs=1, space="PSUM") as ps_otu,
            tc.tile_pool(name="ps_t", bufs=2, space="PSUM") as ps_t,
        ):
            # ------- input DMAs (one descriptor each, parallel queues) ----
            xt = xw.tile([128, NCT, T], BF16, tag="xt", name="xt")
            wqk_sb = xw.tile([128, 3, 2, NCT, 128], BF16, tag="wqk",
                             name="wqk")
            wv_sb = xw.tile([128, NCT, HL * HS], BF16, tag="wv", name="wv")
            wpt_sb = consts.tile([128, 3, C], BF16, tag="wpt", name="wpt")
            # per-pair weight DMAs so scores(0,0) starts as soon as pair 0
            # lands; x in column chunks; three parallel DMA queues
            nc.sync.dma_start(out=wqk_sb[:, 0], in_=wqkp[:, 0])
            nc.scalar.dma_start(out=xt[:, :, 0:512], in_=xh[:, :, 0:512])
            nc.gpsimd.dma_start(out=wv_sb, in_=wvh)
            nc.sync.dma_start(out=wqk_sb[:, 1], in_=wqkp[:, 1])
            nc.scalar.dma_start(out=xt[:, :, 512:1024], in_=xh[:, :, 512:1024])
            nc.sync.dma_start(out=wqk_sb[:, 2], in_=wqkp[:, 2])
            nc.gpsimd.dma_start(out=xt[:, :, 1024:1536],
                                in_=xh[:, :, 1024:1536])
            nc.gpsimd.dma_start(out=wpt_sb, in_=wpth)
            nc.gpsimd.dma_start(out=xt[:, :, 1536:2048],
                                in_=xh[:, :, 1536:2048])

            # warm the PE's HAM clock gate with dummy matmuls while the
            # input DMAs land: ~4-5us of continuous PE activity lifts the
            # clock from 1.2 to 2.4 GHz before the first real matmul
            warm = consts.tile([128, 64], BF16, tag="warm", name="warm")
            nc.gpsimd.memset(warm, 0.0)
            def warm_fill(n):
                # dummy matmuls to keep the PE's HAM activity monitor busy
                # across a known stall so the clock stays at 2.4 GHz
                wps = ps_t.tile([64, 64], F32, tag="tt", name="warmps")
                for _ in range(n):
                    nc.tensor.matmul(wps, warm, warm[:, 0:64],
                                     start=True, stop=True)

            warm_fill(120)

            # ---------------- persistent SBUF tensors --------------------
            vaug = consts.tile([128, NTT, HL, HS + 1], BF16)
            nc.gpsimd.memset(vaug[:, :, :, HS:HS + 1], 1.0)
            pairQ = consts.tile([128, 3, T], BF16, tag="pq", name="pq")
            pairK = consts.tile([128, 3, T], BF16, tag="pk", name="pk")
            otn = consts.tile([128, 3, T], BF16)
            ones_bf = consts.tile([1, HS], BF16)
            nc.gpsimd.memset(ones_bf, 1.0)

            # ---------------- phase subroutines --------------------------
            def qk_pair(m, p):
                sl = slice(m * 512, (m + 1) * 512)
                for qk in range(2):
                    ps = ps_t.tile([128, 512], F32, tag="tt", name="psqk")
                    for ci in range(NCT):
                        nc.tensor.matmul(
                            ps, wqk_sb[:, p, qk, ci, :], xt[:, ci, sl],
                            start=(ci == 0), stop=(ci == NCT - 1),
                        )
                    dst = pairQ if qk == 0 else pairK
                    nc.vector.tensor_copy(out=dst[:, p, sl], in_=ps)

            def qk_chunk(m):
                for p in range(3):
                    qk_pair(m, p)

            def v_chunk(m):
                for tt in range(4 * m, 4 * m + 4):
                    ps = ps_t.tile([128, HL * HS], F32, tag="tt", name="psv")
                    for ci in range(NCT):
                        nc.tensor.matmul(
                            ps, xt[:, ci, tt * 128:(tt + 1) * 128],
                            wv_sb[:, ci, :],
                            start=(ci == 0), stop=(ci == NCT - 1),
                        )
                    nc.vector.tensor_copy(
                        out=vaug[:, tt, :, 0:HS],
                        in_=ps.rearrange("p (h d) -> p h d", h=HL),
                    )

            pts = {}  # (m, p) -> list of pt tiles

            def scores(m, p):
                jmax = 4 * m + 3
                unit = []
                for j in range(jmax + 1):
                    s0 = max(0, j - 4 * m)
                    st = ps_st.tile([128, 2, 512], F32, tag="st", name="st")
                    for e in range(2):
                        nc.tensor.matmul(
                            st[:, e, 128 * s0:512],
                            pairK[64 * e:64 * e + 64, p,
                                  j * 128:(j + 1) * 128],
                            pairQ[64 * e:64 * e + 64, p,
                                  m * 512 + 128 * s0:(m + 1) * 512],
                            start=True, stop=True,
                            tile_position=(64 * e, 0),
                        )
                    pt = ptp.tile([128, 2, 512], BF16, tag="pt", name="pt")
                    unit.append(pt)
                    # one fused exp over both heads (2-bank strided AP)
                    nc.scalar.activation(
                        out=pt[:, :, 128 * s0:512],
                        in_=st[:, :, 128 * s0:512],
                        func=mybir.ActivationFunctionType.Exp,
                        scale=SCALE,
                    )
                    if j >= 4 * m:
                        # zero the below-diagonal triangle of the diagonal
                        # subtile for both heads (keep where tq >= tk)
                        nc.gpsimd.affine_select(
                            out=pt[:, :, 128 * s0:128 * s0 + 128],
                            in_=pt[:, :, 128 * s0:128 * s0 + 128],
                            compare_op=mybir.AluOpType.is_ge,
                            fill=0.0, base=0,
                            pattern=[[0, 2], [1, 128]],
                            channel_multiplier=-1,
                        )
                pts[(m, p)] = unit

            def pv_norm(m, p):
                jmax = 4 * m + 3
                unit = pts.pop((m, p))
                otu_ps = ps_otu.tile([HS + 1, 2, 512], F32, tag="otu",
                                     name="otu")
                for j in range(jmax + 1):
                    s0 = max(0, j - 4 * m)
                    for e in range(2):
                        nc.tensor.matmul(
                            otu_ps[:, e, 128 * s0:512],
                            vaug[:, j, 2 * p + e, :],
                            unit[j][:, e, 128 * s0:512],
                            start=(j == 0), stop=(j == jmax),
                            skip_group_check=True,
                        )
                # normalize: recip of the denominator rows, bf16 broadcast
                # matmul, one multiply per head
                # custom DVE ops ignore nonzero partition bases: copy the
                # denominator row to a partition-0 tile first (tensor_copy
                # does shift partitions; tensor_tensor cannot)
                den0 = small.tile([1, 2, 512], F32, tag="den0", name="den0")
                nc.vector.tensor_copy(out=den0, in_=otu_ps[HS:HS + 1, :, :])
                otu_sb = small.tile([HS + 1, 2, 512], F32, tag="otusb",
                                    name="otusb")
                nc.vector.tensor_copy(out=otu_sb, in_=otu_ps)
                rcp = small.tile([1, 2, 512], F32, tag="rcp", name="rcp")
                nc.vector.reciprocal_approx_fast(out=rcp, in_=den0)
                rcpb = small.tile([1, 2, 512], BF16, tag="rcpb", name="rcpb")
                nc.vector.tensor_copy(out=rcpb, in_=rcp)
                otnorm = small.tile([HS, 2, 512], BF16, tag="otnorm",
                                    name="otnorm")
                for e in range(2):
                    rb = ps_t.tile([HS, 512], F32, tag="tt", name="rb")
                    nc.tensor.matmul(
                        rb, ones_bf[0:1, :], rcpb[0:1, e, :],
                        start=True, stop=True,
                    )
                    nc.vector.tensor_mul(
                        out=otnorm[:, e, :],
                        in0=otu_sb[0:HS, e, :],
                        in1=rb,
                    )
                    nc.sync.dma_start(
                        out=otn[64 * e:64 * e + HS, p,
                                m * 512:(m + 1) * 512],
                        in_=otnorm[:, e, :],
                    )

            def proj(m):
                for tt in range(4 * m, 4 * m + 4):
                    y1 = ps_t.tile([128, 512], F32, tag="tt", name="y1")
                    y2 = ps_t.tile([128, 256], F32, tag="tt", name="y2")
                    for g in range(3):
                        lhs = otn[:, g, tt * 128:(tt + 1) * 128]
                        nc.tensor.matmul(
                            y1, lhs, wpt_sb[:, g, 0:512],
                            start=(g == 0), stop=(g == 2),
                        )
                        nc.tensor.matmul(
                            y2, lhs, wpt_sb[:, g, 512:768],
                            start=(g == 0), stop=(g == 2),
                        )
                    ysb = ysbp.tile([128, C], F32, tag="ysb", name="ysb")
                    nc.vector.tensor_copy(out=ysb[:, 0:512], in_=y1)
                    nc.vector.tensor_copy(out=ysb[:, 512:768], in_=y2)
                    eng = nc.sync if tt % 2 == 0 else nc.scalar
                    eng.dma_start(out=y[tt * 128:(tt + 1) * 128, :], in_=ysb)

            # ---------------- pipelined program order --------------------
            # Units ordered so the ACT-heavy (m=3) exps sit mid-stream with
            # PE fillers (qk/v/proj) around them, and the LAST unit is the
            # smallest (0,2) so the tail is not exp-paced. Scores run one
            # unit ahead of pv (pts pool ring: max 32 tiles alive <= 36).
            qk_pair(0, 0)
            scores(0, 0)
            qk_pair(0, 1)
            scores(0, 1)
            v_chunk(0)
            qk_pair(0, 2)
            pv_norm(0, 0)
            qk_chunk(1)
            scores(1, 0)
            pv_norm(0, 1)
            v_chunk(1)
            scores(1, 1)
            pv_norm(1, 0)
            scores(1, 2)
            pv_norm(1, 1)
            qk_chunk(2)
            scores(2, 0)
            pv_norm(1, 2)
            v_chunk(2)
            scores(2, 1)
            pv_norm(2, 0)
            scores(2, 2)
            pv_norm(2, 1)
            qk_chunk(3)
            scores(3, 0)
            pv_norm(2, 2)
            proj(2)
            v_chunk(3)
            scores(3, 1)
            pv_norm(3, 0)
            scores(0, 2)
            scores(3, 2)
            pv_norm(3, 1)
            proj(1)
            warm_fill(40)
            pv_norm(3, 2)
            proj(3)
            warm_fill(20)
            pv_norm(0, 2)
            proj(0)

    nc.compile()
    return nc


_NC_CACHE = {}


def get_nc(repeat=1, phases=None):
    key = (repeat,)
    if key not in _NC_CACHE:
        nc = bacc.Bacc(
            "TRN2", target_bir_lowering=False, debug=False, num_devices=8
        )
        _NC_CACHE[key] = build_kernel(nc)
    return _NC_CACHE[key]


def make_in_maps(x, Wq, Wk, Wv, Wp):
    x = np.asarray(x, dtype=np.float32)
    Wq = np.asarray(Wq, dtype=np.float32)
    Wk = np.asarray(Wk, dtype=np.float32)
    Wv = np.asarray(Wv, dtype=np.float32)
    Wp = np.asarray(Wp, dtype=np.float32)
    bf = ml_dtypes.bfloat16
    in_maps = []
    for c in range(8):
        b = c // 2
        hs = HL * (c % 2)
        # x: [C, T] -> [128, NCT, T] (partition = row within 128-block)
        xh = np.ascontiguousarray(
            x[b].T.reshape(NCT, 128, T).transpose(1, 0, 2)
        ).astype(bf)
        # pair-stacked Q/K weights: [128, 3(pair), 2(qk), NCT, 128]
        w2 = np.empty((3, 2, NCT, 128, 128), dtype=np.float32)
        for qk, W in enumerate((Wq, Wk)):
            for p in range(3):
                pc = np.concatenate(
                    [W[hs + 2 * p], W[hs + 2 * p + 1]], axis=1
                )  # [C, 128]
                w2[p, qk] = pc.reshape(NCT, 128, 128)
        wqkp = np.ascontiguousarray(w2.transpose(3, 0, 1, 2, 4)).astype(bf)
        # wv: [128, NCT, HL*HS]
        wv_full = np.transpose(Wv[hs:hs + HL], (1, 0, 2)).reshape(C, HL * HS)
        wvh = np.ascontiguousarray(
            wv_full.reshape(NCT, 128, HL * HS).transpose(1, 0, 2)
        ).astype(bf)
        # wpt: Wp[:, i_slice].T -> [384, C] -> [128, 3, C]
        wpth = np.ascontiguousarray(
            Wp[:, hs * HS:(hs + HL) * HS].T.reshape(3, 128, C)
            .transpose(1, 0, 2)
        ).astype(bf)
        in_maps.append({"xh": xh, "wqkp": wqkp, "wvh": wvh, "wpth": wpth})
    return in_maps


def run(x, Wq, Wk, Wv, Wp, bp, trace=False):
    nc = get_nc()
    in_maps = make_in_maps(x, Wq, Wk, Wv, Wp)
    res = bass_utils.run_bass_kernel_spmd(
        nc, in_maps, core_ids=list(range(8)), trace=trace
    )
    y = np.zeros((B, T, C), dtype=np.float32)
    for c in range(8):
        y[c // 2] += res.results[c]["y"]
    y += np.asarray(bp, dtype=np.float32)
    return y, res


def kernel(x, Wq, Wk, Wv, Wp, bp):
    y, _ = run(x, Wq, Wk, Wv, Wp, bp)
    return y


def make_runner(nc):
    """Build the sharded PJRT callable once (mirrors the tail of
    bass2jax.run_bass_via_pjrt) so repeated timed executions don't re-trace.
    Returns (fn, prep) where prep(in_maps) device_puts the inputs and
    fn(device_inputs) -> per-core output dicts (blocking)."""
    import jax
    from jax.experimental.shard_map import shard_map
    from jax.sharding import Mesh, PartitionSpec, NamedSharding
    from concourse import mybir as _mybir
    from concourse.bass2jax import (
        _bass_exec_p, install_neuronx_cc_hook, partition_id_tensor,
    )

    install_neuronx_cc_hook()
    n_cores = 8
    partition_name = (
        nc.partition_id_tensor.name if nc.partition_id_tensor else None
    )
    in_names, out_names, out_avals = [], [], []
    for alloc in nc.m.functions[0].allocations:
        if not isinstance(alloc, _mybir.MemoryLocationSet):
            continue
        name = alloc.memorylocations[0].name
        if alloc.kind == "ExternalInput":
            if name != partition_name:
                in_names.append(name)
        elif alloc.kind == "ExternalOutput":
            out_names.append(name)
            out_avals.append(
                jax.core.ShapedArray(
                    tuple(alloc.tensor_shape), _mybir.dt.np(alloc.dtype)
                )
            )
    n_params = len(in_names)
    n_outs = len(out_avals)
    all_in_names = in_names + out_names
    if partition_name is not None:
        all_in_names.append(partition_name)

    def _body(*args):
        operands = list(args)
        if partition_name is not None:
            operands.append(partition_id_tensor())
        outs = _bass_exec_p.bind(
            *operands,
            out_avals=tuple(out_avals),
            in_names=tuple(all_in_names),
            out_names=tuple(out_names),
            lowering_input_output_aliases=(),
            sim_require_finite=True,
            sim_require_nnan=True,
            nc=nc,
        )
        return tuple(outs)

    devices = jax.devices()[:n_cores]
    mesh = Mesh(np.array(devices), ("core",))
    sharded = jax.jit(
        shard_map(
            _body, mesh=mesh,
            in_specs=(PartitionSpec("core"),) * (n_params + n_outs),
            out_specs=(PartitionSpec("core"),) * n_outs,
            check_rep=False,
        ),
        donate_argnums=tuple(range(n_params, n_params + n_outs)),
        keep_unused=True,
    )
    shd = NamedSharding(mesh, PartitionSpec("core"))

    def prep(in_maps):
        return [
            jax.device_put(
                np.concatenate([in_maps[c][nm] for c in range(n_cores)], axis=0),
                shd,
            )
            for nm in in_names
        ]

    def zeros():
        return [
            jax.device_put(
                np.zeros((n_cores * a.shape[0], *a.shape[1:]), a.dtype), shd
            )
            for a in out_avals
        ]

    def fn(dev_inputs, dev_zeros):
        outs = sharded(*dev_inputs, *dev_zeros)
        jax.block_until_ready(outs)
        return outs

    def make_loop_fn(n_iters):
        def _body_n(*args):
            ins = args[:n_params]
            carry = tuple(args[n_params:])

            def step(i, carry):
                operands = list(ins) + list(carry)
                if partition_name is not None:
                    operands.append(partition_id_tensor())
                outs = _bass_exec_p.bind(
                    *operands,
                    out_avals=tuple(out_avals),
                    in_names=tuple(all_in_names),
                    out_names=tuple(out_names),
                    lowering_input_output_aliases=(),
                    sim_require_finite=True,
                    sim_require_nnan=True,
                    nc=nc,
                )
                return tuple(outs)

            return jax.lax.fori_loop(0, n_iters, step, carry)

        looped = jax.jit(
            shard_map(
                _body_n, mesh=mesh,
                in_specs=(PartitionSpec("core"),) * (n_params + n_outs),
                out_specs=(PartitionSpec("core"),) * n_outs,
                check_rep=False,
            ),
            donate_argnums=tuple(range(n_params, n_params + n_outs)),
            keep_unused=True,
        )

        def run_n(dev_inputs, dev_zeros):
            outs = looped(*dev_inputs, *dev_zeros)
            jax.block_until_ready(outs)
            return outs

        return run_n

    return fn, prep, zeros, out_names, make_loop_fn


# revision 25
# speedup vs baseline: 1.0408x; 1.0408x over previous
"""Multi-head causal attention (B=4, T=2048, C=768, H=12, HS=64) on 8 trn2 cores.

Sharding: 48 (batch, head) units -> 6 per core. Core c: batch c//2, heads
6*(c%2) .. 6*(c%2)+6. Each core computes a partial output projection
y_partial[T, C] = sum over its 6 heads; host sums the two partials per batch
and adds the bias.

Per-core design (v2 — software-pipelined):
  All inputs are packed on the host in their final SBUF layout (leading 128
  partition dim) so each lands with ONE dma descriptor; x is loaded in four
  512-column chunks so compute starts after ~2 descriptors.

  Q/K weights are pair-stacked on the host (head 2p cols 0:64, head 2p+1
  cols 64:128), so the QK projection psum IS the pair-stacked pairQ/pairK
  layout and a single DVE copy per (p, qk, chunk) replaces the staging
  SBUF->SBUF DMAs.

  scores  ST[tk, tq] = matmul(lhsT=pairK 64-rows, rhs=pairQ 64-rows) with
          tile_position=(64e, 0): both heads of a pair run concurrently in
          disjoint PE row groups.
  softmax no max-subtraction (scores are O(+-8)); exp on ACT (bf16 out);
          row sums ride along as a ones-column appended to V (vaug col 64).
  P@V     OTu[65, tq] accumulated over tk tiles; row 64 = denominator.
  norm    reciprocal_approx_fast on the [1, 2, 512] denominator rows,
          bf16 K=1 broadcast matmul (fast; the fp32 one costs 1.1us/MM),
          one DVE multiply, DMA into pair-stacked otn.
  proj    y[tq, :] = sum_g matmul(lhsT=otn[:, g, tq], rhs=wpt[g]).

  Program order interleaves the 12 (m-chunk, pair) attention units with the
  projection chunks so the PE never idles >3us (HAM stays at K=8/8) and the
  ACT engine (exp: the co-bottleneck at ~120us) is fed continuously.
"""

import numpy as np
import ml_dtypes

import concourse.bacc as bacc
import concourse.bass as bass
import concourse.tile as tile
from concourse import mybir
from concourse import bass_utils

B, T, C = 4, 2048, 768
H, HS = 12, 64
HL = 6            # heads per core
NCT = C // 128    # 6 contraction tiles
NTT = T // 128    # 16 t tiles
NTC = T // 512    # 4 t chunks
SCALE = 1.0 / 8.0  # 1/sqrt(HS)

F32 = mybir.dt.float32
BF16 = mybir.dt.bfloat16


def build_kernel(nc):
    xh = nc.dram_tensor("xh", [128, NCT, T], BF16, kind="ExternalInput").ap()
    wqkp = nc.dram_tensor("wqkp", [128, 3, 2, NCT, 128], BF16,
                          kind="ExternalInput").ap()
    wvh = nc.dram_tensor("wvh", [128, NCT, HL * HS], BF16,
                         kind="ExternalInput").ap()
    wpth = nc.dram_tensor("wpth", [128, 3, C], BF16, kind="ExternalInput").ap()
    y = nc.dram_tensor("y", [T, C], F32, kind="ExternalOutput").ap()

    with tile.TileContext(nc) as tc:
        with (
            tc.tile_pool(name="consts", bufs=1) as consts,
            tc.tile_pool(name="xw", bufs=1) as xw,
            tc.tile_pool(name="pt", bufs=36) as ptp,
            tc.tile_pool(name="small", bufs=2) as small,
            tc.tile_pool(name="ysb", bufs=2) as ysbp,
            # PSUM: st 2x2 banks + otu 1x2 banks + tt 2x1 banks = 8 banks
            tc.tile_pool(name="ps_st", bufs=2, space="PSUM") as ps_st,
            tc.tile_pool(name="ps_otu", bufs=1, space="PSUM") as ps_otu,
            tc.tile_pool(name="ps_t", bufs=2, space="PSUM") as ps_t,
        ):
            # ------- input DMAs (one descriptor each, parallel queues) ----
            xt = xw.tile([128, NCT, T], BF16, tag="xt", name="xt")
            wqk_sb = xw.tile([128, 3, 2, NCT, 128], BF16, tag="wqk",
                             name="wqk")
            wv_sb = xw.tile([128, NCT, HL * HS], BF16, tag="wv", name="wv")
            wpt_sb = consts.tile([128, 3, C], BF16, tag="wpt", name="wpt")
            # per-pair weight DMAs so scores(0,0) starts as soon as pair 0
            # lands; x in column chunks; three parallel DMA queues
            nc.sync.dma_start(out=wqk_sb[:, 0], in_=wqkp[:, 0])
            nc.scalar.dma_start(out=xt[:, :, 0:512], in_=xh[:, :, 0:512])
            nc.gpsimd.dma_start(out=wv_sb, in_=wvh)
            nc.sync.dma_start(out=wqk_sb[:, 1], in_=wqkp[:, 1])
            nc.scalar.dma_start(out=xt[:, :, 512:1024], in_=xh[:, :, 512:1024])
            nc.sync.dma_start(out=wqk_sb[:, 2], in_=wqkp[:, 2])
            nc.gpsimd.dma_start(out=xt[:, :, 1024:1536],
                                in_=xh[:, :, 1024:1536])
            nc.gpsimd.dma_start(out=wpt_sb, in_=wpth)
            nc.gpsimd.dma_start(out=xt[:, :, 1536:2048],
                                in_=xh[:, :, 1536:2048])

            # warm the PE's HAM clock gate with dummy matmuls while the
            # input DMAs land: ~4-5us of continuous PE activity lifts the
            # clock from 1.2 to 2.4 GHz before the first real matmul
            warm = consts.tile([128, 64], BF16, tag="warm", name="warm")
            nc.gpsimd.memset(warm, 0.0)
            def warm_fill(n):
                # dummy matmuls to keep the PE's HAM activity monitor busy
                # across a known stall so the clock stays at 2.4 GHz
                wps = ps_t.tile([64, 64], F32, tag="tt", name="warmps")
                for _ in range(n):
                    nc.tensor.matmul(wps, warm, warm[:, 0:64],
                                     start=True, stop=True)

            warm_fill(120)

            # ---------------- persistent SBUF tensors --------------------
            vaug = consts.tile([128, NTT, HL, HS + 1], BF16)
            nc.gpsimd.memset(vaug[:, :, :, HS:HS + 1], 1.0)
            pairQ = consts.tile([128, 3, T], BF16, tag="pq", name="pq")
            pairK = consts.tile([128, 3, T], BF16, tag="pk", name="pk")
            otn = consts.tile([128, 3, T], BF16)
            ones_bf = consts.tile([1, HS], BF16)
            nc.gpsimd.memset(ones_bf, 1.0)

            # ---------------- phase subroutines --------------------------
            def qk_pair(m, p):
                sl = slice(m * 512, (m + 1) * 512)
                for qk in range(2):
                    ps = ps_t.tile([128, 512], F32, tag="tt", name="psqk")
                    for ci in range(NCT):
                        nc.tensor.matmul(
                            ps, wqk_sb[:, p, qk, ci, :], xt[:, ci, sl],
                            start=(ci == 0), stop=(ci == NCT - 1),
                        )
                    dst = pairQ if qk == 0 else pairK
                    nc.vector.tensor_copy(out=dst[:, p, sl], in_=ps)

            def qk_chunk(m):
                for p in range(3):
                    qk_pair(m, p)

            def v_chunk(m):
                for tt in range(4 * m, 4 * m + 4):
                    ps = ps_t.tile([128, HL * HS], F32, tag="tt", name="psv")
                    for ci in range(NCT):
                        nc.tensor.matmul(
                            ps, xt[:, ci, tt * 128:(tt + 1) * 128],
                            wv_sb[:, ci, :],
                            start=(ci == 0), stop=(ci == NCT - 1),
                        )
                    nc.vector.tensor_copy(
                        out=vaug[:, tt, :, 0:HS],
                        in_=ps.rearrange("p (h d) -> p h d", h=HL),
                    )

            pts = {}  # (m, p) -> list of pt tiles

            def scores_j(m, p, j, unit):
                s0 = max(0, j - 4 * m)
                st = ps_st.tile([128, 2, 512], F32, tag="st", name="st")
                for e in range(2):
                    nc.tensor.matmul(
                        st[:, e, 128 * s0:512],
                        pairK[64 * e:64 * e + 64, p,
                              j * 128:(j + 1) * 128],
                        pairQ[64 * e:64 * e + 64, p,
                              m * 512 + 128 * s0:(m + 1) * 512],
                        start=True, stop=True,
                        tile_position=(64 * e, 0),
                    )
                pt = ptp.tile([128, 2, 512], BF16, tag="pt", name="pt")
                unit.append(pt)
                # one fused exp over both heads (2-bank strided AP)
                nc.scalar.activation(
                    out=pt[:, :, 128 * s0:512],
                    in_=st[:, :, 128 * s0:512],
                    func=mybir.ActivationFunctionType.Exp,
                    scale=SCALE,
                )
                if j >= 4 * m:
                    # zero the below-diagonal triangle of the diagonal
                    # subtile for both heads (keep where tq >= tk)
                    nc.gpsimd.affine_select(
                        out=pt[:, :, 128 * s0:128 * s0 + 128],
                        in_=pt[:, :, 128 * s0:128 * s0 + 128],
                        compare_op=mybir.AluOpType.is_ge,
                        fill=0.0, base=0,
                        pattern=[[0, 2], [1, 128]],
                        channel_multiplier=-1,
                    )

            def pv_j(m, p, j, unit, otu_ps):
                s0 = max(0, j - 4 * m)
                jmax = 4 * m + 3
                for e in range(2):
                    nc.tensor.matmul(
                        otu_ps[:, e, 128 * s0:512],
                        vaug[:, j, 2 * p + e, :],
                        unit[j][:, e, 128 * s0:512],
                        start=(j == 0), stop=(j == jmax),
                        skip_group_check=True,
                    )

            def scores(m, p):
                unit = []
                for j in range(4 * m + 4):
                    scores_j(m, p, j, unit)
                pts[(m, p)] = unit

            def fused(su, pu):
                """Interleave scores unit su's j-loop with pv unit pu's:
                the pv matmuls fill the PE while exp drains the st ring."""
                ms, ps_ = su
                mp, pp = pu
                s_unit = []
                p_unit = pts.pop(pu)
                otu_ps = ps_otu.tile([HS + 1, 2, 512], F32, tag="otu",
                                     name="otu")
                for j in range(max(4 * ms + 4, 4 * mp + 4)):
                    if j < 4 * ms + 4:
                        scores_j(ms, ps_, j, s_unit)
                    if j < 4 * mp + 4:
                        pv_j(mp, pp, j, p_unit, otu_ps)
                pts[su] = s_unit
                norm(pu, otu_ps)

            def pv_norm(m, p):
                unit = pts.pop((m, p))
                otu_ps = ps_otu.tile([HS + 1, 2, 512], F32, tag="otu",
                                     name="otu")
                for j in range(4 * m + 4):
                    pv_j(m, p, j, unit, otu_ps)
                norm((m, p), otu_ps)

            def norm(pu, otu_ps):
                # normalize: recip of the denominator rows, bf16 broadcast
                # matmul, one multiply per head
                m, p = pu
                # custom DVE ops ignore nonzero partition bases: copy the
                # denominator row to a partition-0 tile first (tensor_copy
                # does shift partitions; tensor_tensor cannot)
                den0 = small.tile([1, 2, 512], F32, tag="den0", name="den0")
                nc.vector.tensor_copy(out=den0, in_=otu_ps[HS:HS + 1, :, :])
                otu_sb = small.tile([HS + 1, 2, 512], F32, tag="otusb",
                                    name="otusb")
                nc.vector.tensor_copy(out=otu_sb, in_=otu_ps)
                rcp = small.tile([1, 2, 512], F32, tag="rcp", name="rcp")
                nc.vector.reciprocal_approx_fast(out=rcp, in_=den0)
                rcpb = small.tile([1, 2, 512], BF16, tag="rcpb", name="rcpb")
                nc.vector.tensor_copy(out=rcpb, in_=rcp)
                otnorm = small.tile([HS, 2, 512], BF16, tag="otnorm",
                                    name="otnorm")
                for e in range(2):
                    rb = ps_t.tile([HS, 512], F32, tag="tt", name="rb")
                    nc.tensor.matmul(
                        rb, ones_bf[0:1, :], rcpb[0:1, e, :],
                        start=True, stop=True,
                    )
                    nc.vector.tensor_mul(
                        out=otnorm[:, e, :],
                        in0=otu_sb[0:HS, e, :],
                        in1=rb,
                    )
                    nc.sync.dma_start(
                        out=otn[64 * e:64 * e + HS, p,
                                m * 512:(m + 1) * 512],
                        in_=otnorm[:, e, :],
                    )

            def proj(m):
                for tt in range(4 * m, 4 * m + 4):
                    y1 = ps_t.tile([128, 512], F32, tag="tt", name="y1")
                    y2 = ps_t.tile([128, 256], F32, tag="tt", name="y2")
                    for g in range(3):
                        lhs = otn[:, g, tt * 128:(tt + 1) * 128]
                        nc.tensor.matmul(
                            y1, lhs, wpt_sb[:, g, 0:512],
                            start=(g == 0), stop=(g == 2),
                        )
                        nc.tensor.matmul(
                            y2, lhs, wpt_sb[:, g, 512:768],
                            start=(g == 0), stop=(g == 2),
                        )
                    ysb = ysbp.tile([128, C], F32, tag="ysb", name="ysb")
                    nc.vector.tensor_copy(out=ysb[:, 0:512], in_=y1)
                    nc.vector.tensor_copy(out=ysb[:, 512:768], in_=y2)
                    eng = nc.sync if tt % 2 == 0 else nc.scalar
                    eng.dma_start(out=y[tt * 128:(tt + 1) * 128, :], in_=ysb)

            # ---------------- pipelined program order --------------------
            # Units ordered so the ACT-heavy (m=3) exps sit mid-stream with
            # PE fillers (qk/v/proj) around them, and the LAST unit is the
            # smallest (0,2) so the tail is not exp-paced. Scores run one
            # unit ahead of pv (pts pool ring: max 32 tiles alive <= 36).
            qk_pair(0, 0)
            scores(0, 0)
            qk_pair(0, 1)
            v_chunk(0)
            fused((0, 1), (0, 0))
            qk_pair(0, 2)
            qk_pair(1, 0)
            fused((1, 0), (0, 1))
            qk_pair(1, 1)
            v_chunk(1)
            fused((1, 1), (1, 0))
            qk_pair(1, 2)
            fused((1, 2), (1, 1))
            qk_pair(2, 0)
            fused((2, 0), (1, 2))
            qk_pair(2, 1)
            v_chunk(2)
            fused((2, 1), (2, 0))
            qk_pair(2, 2)
            fused((2, 2), (2, 1))
            qk_pair(3, 0)
            fused((3, 0), (2, 2))
            proj(2)
            qk_pair(3, 1)
            v_chunk(3)
            fused((3, 1), (3, 0))
            qk_pair(3, 2)
            fused((3, 2), (3, 1))
            proj(1)
            fused((0, 2), (3, 2))
            proj(3)
            pv_norm(0, 2)
            proj(0)

    nc.compile()
    return nc


_NC_CACHE = {}


def get_nc(repeat=1, phases=None):
    key = (repeat,)
    if key not in _NC_CACHE:
        nc = bacc.Bacc(
            "TRN2", target_bir_lowering=False, debug=False, num_devices=8
        )
        _NC_CACHE[key] = build_kernel(nc)
    return _NC_CACHE[key]


def make_in_maps(x, Wq, Wk, Wv, Wp):
    x = np.asarray(x, dtype=np.float32)
    Wq = np.asarray(Wq, dtype=np.float32)
    Wk = np.asarray(Wk, dtype=np.float32)
    Wv = np.asarray(Wv, dtype=np.float32)
    Wp = np.asarray(Wp, dtype=np.float32)
    bf = ml_dtypes.bfloat16
    in_maps = []
    for c in range(8):
        b = c // 2
        hs = HL * (c % 2)
        # x: [C, T] -> [128, NCT, T] (partition = row within 128-block)
        xh = np.ascontiguousarray(
            x[b].T.reshape(NCT, 128, T).transpose(1, 0, 2)
        ).astype(bf)
        # pair-stacked Q/K weights: [128, 3(pair), 2(qk), NCT, 128]
        w2 = np.empty((3, 2, NCT, 128, 128), dtype=np.float32)
        for qk, W in enumerate((Wq, Wk)):
            for p in range(3):
                pc = np.concatenate(
                    [W[hs + 2 * p], W[hs + 2 * p + 1]], axis=1
                )  # [C, 128]
                w2[p, qk] = pc.reshape(NCT, 128, 128)
        wqkp = np.ascontiguousarray(w2.transpose(3, 0, 1, 2, 4)).astype(bf)
        # wv: [128, NCT, HL*HS]
        wv_full = np.transpose(Wv[hs:hs + HL], (1, 0, 2)).reshape(C, HL * HS)
        wvh = np.ascontiguousarray(
            wv_full.reshape(NCT, 128, HL * HS).transpose(1, 0, 2)
        ).astype(bf)
        # wpt: Wp[:, i_slice].T -> [384, C] -> [128, 3, C]
        wpth = np.ascontiguousarray(
            Wp[:, hs * HS:(hs + HL) * HS].T.reshape(3, 128, C)
            .transpose(1, 0, 2)
        ).astype(bf)
        in_maps.append({"xh": xh, "wqkp": wqkp, "wvh": wvh, "wpth": wpth})
    return in_maps


def run(x, Wq, Wk, Wv, Wp, bp, trace=False):
    nc = get_nc()
    in_maps = make_in_maps(x, Wq, Wk, Wv, Wp)
    res = bass_utils.run_bass_kernel_spmd(
        nc, in_maps, core_ids=list(range(8)), trace=trace
    )
    y = np.zeros((B, T, C), dtype=np.float32)
    for c in range(8):
        y[c // 2] += res.results[c]["y"]
    y += np.asarray(bp, dtype=np.float32)
    return y, res


def kernel(x, Wq, Wk, Wv, Wp, bp):
    y, _ = run(x, Wq, Wk, Wv, Wp, bp)
    return y


def make_runner(nc):
    """Build the sharded PJRT callable once (mirrors the tail of
    bass2jax.run_bass_via_pjrt) so repeated timed executions don't re-trace.
    Returns (fn, prep) where prep(in_maps) device_puts the inputs and
    fn(device_inputs) -> per-core output dicts (blocking)."""
    import jax
    from jax.experimental.shard_map import shard_map
    from jax.sharding import Mesh, PartitionSpec, NamedSharding
    from concourse import mybir as _mybir
    from concourse.bass2jax import (
        _bass_exec_p, install_neuronx_cc_hook, partition_id_tensor,
    )

    install_neuronx_cc_hook()
    n_cores = 8
    partition_name = (
        nc.partition_id_tensor.name if nc.partition_id_tensor else None
    )
    in_names, out_names, out_avals = [], [], []
    for alloc in nc.m.functions[0].allocations:
        if not isinstance(alloc, _mybir.MemoryLocationSet):
            continue
        name = alloc.memorylocations[0].name
        if alloc.kind == "ExternalInput":
            if name != partition_name:
                in_names.append(name)
        elif alloc.kind == "ExternalOutput":
            out_names.append(name)
            out_avals.append(
                jax.core.ShapedArray(
                    tuple(alloc.tensor_shape), _mybir.dt.np(alloc.dtype)
                )
            )
    n_params = len(in_names)
    n_outs = len(out_avals)
    all_in_names = in_names + out_names
    if partition_name is not None:
        all_in_names.append(partition_name)

    def _body(*args):
        operands = list(args)
        if partition_name is not None:
            operands.append(partition_id_tensor())
        outs = _bass_exec_p.bind(
            *operands,
            out_avals=tuple(out_avals),
            in_names=tuple(all_in_names),
            out_names=tuple(out_names),
            lowering_input_output_aliases=(),
            sim_require_finite=True,
            sim_require_nnan=True,
            nc=nc,
        )
        return tuple(outs)

    devices = jax.devices()[:n_cores]
    mesh = Mesh(np.array(devices), ("core",))
    sharded = jax.jit(
        shard_map(
            _body, mesh=mesh,
            in_specs=(PartitionSpec("core"),) * (n_params + n_outs),
            out_specs=(PartitionSpec("core"),) * n_outs,
            check_rep=False,
        ),
        donate_argnums=tuple(range(n_params, n_params + n_outs)),
        keep_unused=True,
    )
    shd = NamedSharding(mesh, PartitionSpec("core"))

    def prep(in_maps):
        return [
            jax.device_put(
                np.concatenate([in_maps[c][nm] for c in range(n_cores)], axis=0),
                shd,
            )
            for nm in in_names
        ]

    def zeros():
        return [
            jax.device_put(
                np.zeros((n_cores * a.shape[0], *a.shape[1:]), a.dtype), shd
            )
            for a in out_avals
        ]

    def fn(dev_inputs, dev_zeros):
        outs = sharded(*dev_inputs, *dev_zeros)
        jax.block_until_ready(outs)
        return outs

    def make_loop_fn(n_iters):
        def _body_n(*args):
            ins = args[:n_params]
            carry = tuple(args[n_params:])

            def step(i, carry):
                operands = list(ins) + list(carry)
                if partition_name is not None:
                    operands.append(partition_id_tensor())
                outs = _bass_exec_p.bind(
                    *operands,
                    out_avals=tuple(out_avals),
                    in_names=tuple(all_in_names),
                    out_names=tuple(out_names),
                    lowering_input_output_aliases=(),
                    sim_require_finite=True,
                    sim_require_nnan=True,
                    nc=nc,
                )
                return tuple(outs)

            return jax.lax.fori_loop(0, n_iters, step, carry)

        looped = jax.jit(
            shard_map(
                _body_n, mesh=mesh,
                in_specs=(PartitionSpec("core"),) * (n_params + n_outs),
                out_specs=(PartitionSpec("core"),) * n_outs,
                check_rep=False,
            ),
            donate_argnums=tuple(range(n_params, n_params + n_outs)),
            keep_unused=True,
        )

        def run_n(dev_inputs, dev_zeros):
            outs = looped(*dev_inputs, *dev_zeros)
            jax.block_until_ready(outs)
            return outs

        return run_n

    return fn, prep, zeros, out_names, make_loop_fn


# revision 30
# speedup vs baseline: 1.1139x; 1.0702x over previous
"""Multi-head causal attention (B=4, T=2048, C=768, H=12, HS=64) on 8 trn2 cores.

Sharding: 48 (batch, head) units -> 6 per core. Core c: batch c//2, heads
6*(c%2) .. 6*(c%2)+6. Each core computes a partial output projection
y_partial[T, C] = sum over its 6 heads; host sums the two partials per batch
and adds the bias.

Per-core design (v2 — software-pipelined):
  All inputs are packed on the host in their final SBUF layout (leading 128
  partition dim) so each lands with ONE dma descriptor; x is loaded in four
  512-column chunks so compute starts after ~2 descriptors.

  Q/K weights are pair-stacked on the host (head 2p cols 0:64, head 2p+1
  cols 64:128), so the QK projection psum IS the pair-stacked pairQ/pairK
  layout and a single DVE copy per (p, qk, chunk) replaces the staging
  SBUF->SBUF DMAs.

  scores  ST[tk, tq] = matmul(lhsT=pairK 64-rows, rhs=pairQ 64-rows) with
          tile_position=(64e, 0): both heads of a pair run concurrently in
          disjoint PE row groups.
  softmax no max-subtraction (scores are O(+-8)); exp on ACT (bf16 out);
          row sums ride along as a ones-column appended to V (vaug col 64).
  P@V     OTu[65, tq] accumulated over tk tiles; row 64 = denominator.
  norm    reciprocal_approx_fast on the [1, 2, 512] denominator rows,
          bf16 K=1 broadcast matmul (fast; the fp32 one costs 1.1us/MM),
          one DVE multiply, DMA into pair-stacked otn.
  proj    y[tq, :] = sum_g matmul(lhsT=otn[:, g, tq], rhs=wpt[g]).

  Program order interleaves the 12 (m-chunk, pair) attention units with the
  projection chunks so the PE never idles >3us (HAM stays at K=8/8) and the
  ACT engine (exp: the co-bottleneck at ~120us) is fed continuously.
"""

import numpy as np
import ml_dtypes

import concourse.bacc as bacc
import concourse.bass as bass
import concourse.tile as tile
from concourse import mybir
from concourse import bass_utils

B, T, C = 4, 2048, 768
H, HS = 12, 64
HL = 6            # heads per core
NCT = C // 128    # 6 contraction tiles
NTT = T // 128    # 16 t tiles
NTC = T // 512    # 4 t chunks
SCALE = 1.0 / 8.0  # 1/sqrt(HS)

F32 = mybir.dt.float32
BF16 = mybir.dt.bfloat16


def build_kernel(nc):
    xh = nc.dram_tensor("xh", [128, NCT, T], BF16, kind="ExternalInput").ap()
    wqkp = nc.dram_tensor("wqkp", [128, 3, 2, NCT, 128], BF16,
                          kind="ExternalInput").ap()
    wvh = nc.dram_tensor("wvh", [128, NCT, HL * HS], BF16,
                         kind="ExternalInput").ap()
    wpth = nc.dram_tensor("wpth", [128, 3, C], BF16, kind="ExternalInput").ap()
    y = nc.dram_tensor("y", [T, C], F32, kind="ExternalOutput").ap()

    with tile.TileContext(nc) as tc:
        with (
            tc.tile_pool(name="consts", bufs=1) as consts,
            tc.tile_pool(name="xw", bufs=1) as xw,
            tc.tile_pool(name="pt", bufs=36) as ptp,
            tc.tile_pool(name="small", bufs=2) as small,
            tc.tile_pool(name="ysb", bufs=2) as ysbp,
            # PSUM: st 2x2 banks + otu 1x2 banks + tt 2x1 banks = 8 banks
            tc.tile_pool(name="ps_st", bufs=2, space="PSUM") as ps_st,
            tc.tile_pool(name="ps_otu", bufs=1, space="PSUM") as ps_otu,
            tc.tile_pool(name="ps_t", bufs=2, space="PSUM") as ps_t,
        ):
            # ------- input DMAs (one descriptor each, parallel queues) ----
            xt = xw.tile([128, NCT, T], BF16, tag="xt", name="xt")
            wqk_sb = xw.tile([128, 3, 2, NCT, 128], BF16, tag="wqk",
                             name="wqk")
            wv_sb = xw.tile([128, NCT, HL * HS], BF16, tag="wv", name="wv")
            wpt_sb = consts.tile([128, 3, C], BF16, tag="wpt", name="wpt")
            # per-pair weight DMAs so scores(0,0) starts as soon as pair 0
            # lands; x in column chunks; three parallel DMA queues
            nc.sync.dma_start(out=wqk_sb[:, 0], in_=wqkp[:, 0])
            nc.scalar.dma_start(out=xt[:, :, 0:512], in_=xh[:, :, 0:512])
            nc.gpsimd.dma_start(out=wv_sb, in_=wvh)
            nc.sync.dma_start(out=wqk_sb[:, 1], in_=wqkp[:, 1])
            nc.scalar.dma_start(out=xt[:, :, 512:1024], in_=xh[:, :, 512:1024])
            nc.sync.dma_start(out=wqk_sb[:, 2], in_=wqkp[:, 2])
            nc.gpsimd.dma_start(out=xt[:, :, 1024:1536],
                                in_=xh[:, :, 1024:1536])
            nc.gpsimd.dma_start(out=wpt_sb, in_=wpth)
            nc.gpsimd.dma_start(out=xt[:, :, 1536:2048],
                                in_=xh[:, :, 1536:2048])

            # warm the PE's HAM clock gate with dummy matmuls while the
            # input DMAs land: ~4-5us of continuous PE activity lifts the
            # clock from 1.2 to 2.4 GHz before the first real matmul
            warm = consts.tile([128, 64], BF16, tag="warm", name="warm")
            nc.gpsimd.memset(warm, 0.0)
            def warm_fill(n):
                # dummy matmuls to keep the PE's HAM activity monitor busy
                # across a known stall so the clock stays at 2.4 GHz
                wps = ps_t.tile([64, 64], F32, tag="tt", name="warmps")
                for _ in range(n):
                    nc.tensor.matmul(wps, warm, warm[:, 0:64],
                                     start=True, stop=True)

            warm_fill(120)

            # ---------------- persistent SBUF tensors --------------------
            vaug = consts.tile([128, NTT, HL, HS + 1], BF16)
            nc.gpsimd.memset(vaug[:, :, :, HS:HS + 1], 1.0)
            pairQ = consts.tile([128, 3, T], BF16, tag="pq", name="pq")
            pairK = consts.tile([128, 3, T], BF16, tag="pk", name="pk")
            otn = consts.tile([128, 3, T], BF16)
            ones_bf = consts.tile([1, HS], BF16)
            nc.gpsimd.memset(ones_bf, 1.0)

            # ---------------- phase subroutines --------------------------
            def qk_half(m, p, qk):
                sl = slice(m * 512, (m + 1) * 512)
                ps = ps_t.tile([128, 512], F32, tag="tt", name="psqk")
                for ci in range(NCT):
                    nc.tensor.matmul(
                        ps, wqk_sb[:, p, qk, ci, :], xt[:, ci, sl],
                        start=(ci == 0), stop=(ci == NCT - 1),
                    )
                dst = pairQ if qk == 0 else pairK
                nc.vector.tensor_copy(out=dst[:, p, sl], in_=ps)

            def qk_pair(m, p):
                qk_half(m, p, 0)
                qk_half(m, p, 1)

            def v_tile(tt):
                ps = ps_t.tile([128, HL * HS], F32, tag="tt", name="psv")
                for ci in range(NCT):
                    nc.tensor.matmul(
                        ps, xt[:, ci, tt * 128:(tt + 1) * 128],
                        wv_sb[:, ci, :],
                        start=(ci == 0), stop=(ci == NCT - 1),
                    )
                nc.vector.tensor_copy(
                    out=vaug[:, tt, :, 0:HS],
                    in_=ps.rearrange("p (h d) -> p h d", h=HL),
                )

            def v_chunk(m):
                for tt in range(4 * m, 4 * m + 4):
                    v_tile(tt)

            pts = {}  # (m, p) -> list of pt tiles

            def scores_j(m, p, j, unit):
                s0 = max(0, j - 4 * m)
                st = ps_st.tile([128, 2, 512], F32, tag="st", name="st")
                for e in range(2):
                    nc.tensor.matmul(
                        st[:, e, 128 * s0:512],
                        pairK[64 * e:64 * e + 64, p,
                              j * 128:(j + 1) * 128],
                        pairQ[64 * e:64 * e + 64, p,
                              m * 512 + 128 * s0:(m + 1) * 512],
                        start=True, stop=True,
                        tile_position=(64 * e, 0),
                    )
                pt = ptp.tile([128, 2, 512], BF16, tag="pt", name="pt")
                unit.append(pt)
                # one fused exp over both heads (2-bank strided AP)
                nc.scalar.activation(
                    out=pt[:, :, 128 * s0:512],
                    in_=st[:, :, 128 * s0:512],
                    func=mybir.ActivationFunctionType.Exp,
                    scale=SCALE,
                )
                if j >= 4 * m:
                    # zero the below-diagonal triangle of the diagonal
                    # subtile for both heads (keep where tq >= tk)
                    nc.gpsimd.affine_select(
                        out=pt[:, :, 128 * s0:128 * s0 + 128],
                        in_=pt[:, :, 128 * s0:128 * s0 + 128],
                        compare_op=mybir.AluOpType.is_ge,
                        fill=0.0, base=0,
                        pattern=[[0, 2], [1, 128]],
                        channel_multiplier=-1,
                    )

            def pv_j(m, p, j, unit, otu_ps):
                s0 = max(0, j - 4 * m)
                jmax = 4 * m + 3
                for e in range(2):
                    nc.tensor.matmul(
                        otu_ps[:, e, 128 * s0:512],
                        vaug[:, j, 2 * p + e, :],
                        unit[j][:, e, 128 * s0:512],
                        start=(j == 0), stop=(j == jmax),
                        skip_group_check=True,
                    )

            def scores(m, p):
                unit = []
                for j in range(4 * m + 4):
                    scores_j(m, p, j, unit)
                pts[(m, p)] = unit

            def fused(su, pu, fillers=()):
                """Interleave scores unit su's j-loop with pv unit pu's;
                `fillers` (closures of ~1us of PE work) are emitted at paced
                j positions so the PE stays busy while ACT drains the st
                ring (exp is slower per tile than the score matmuls)."""
                ms, ps_ = su
                s_unit = []
                if pu is not None:
                    mp, pp = pu
                    p_unit = pts.pop(pu)
                    otu_ps = ps_otu.tile([HS + 1, 2, 512], F32, tag="otu",
                                         name="otu")
                nj = 4 * ms + 4
                fillers = list(fillers)
                nf = len(fillers)
                fired = 0
                for j in range(max(nj, 4 * mp + 4 if pu is not None else 0)):
                    if j < nj:
                        scores_j(ms, ps_, j, s_unit)
                    if pu is not None and j < 4 * mp + 4:
                        pv_j(mp, pp, j, p_unit, otu_ps)
                    while fired < nf * (j + 1) // max(nj, 1):
                        fillers[fired]()
                        fired += 1
                for f in fillers[fired:]:
                    f()
                pts[su] = s_unit
                if pu is not None:
                    norm(pu, otu_ps)

            def pv_norm(m, p):
                unit = pts.pop((m, p))
                otu_ps = ps_otu.tile([HS + 1, 2, 512], F32, tag="otu",
                                     name="otu")
                for j in range(4 * m + 4):
                    pv_j(m, p, j, unit, otu_ps)
                norm((m, p), otu_ps)

            def norm(pu, otu_ps):
                # normalize: recip of the denominator rows, bf16 broadcast
                # matmul, one multiply per head
                m, p = pu
                # custom DVE ops ignore nonzero partition bases: copy the
                # denominator row to a partition-0 tile first (tensor_copy
                # does shift partitions; tensor_tensor cannot)
                den0 = small.tile([1, 2, 512], F32, tag="den0", name="den0")
                nc.vector.tensor_copy(out=den0, in_=otu_ps[HS:HS + 1, :, :])
                otu_sb = small.tile([HS + 1, 2, 512], F32, tag="otusb",
                                    name="otusb")
                nc.vector.tensor_copy(out=otu_sb, in_=otu_ps)
                rcp = small.tile([1, 2, 512], F32, tag="rcp", name="rcp")
                nc.vector.reciprocal_approx_fast(out=rcp, in_=den0)
                rcpb = small.tile([1, 2, 512], BF16, tag="rcpb", name="rcpb")
                nc.vector.tensor_copy(out=rcpb, in_=rcp)
                otnorm = small.tile([HS, 2, 512], BF16, tag="otnorm",
                                    name="otnorm")
                for e in range(2):
                    rb = ps_t.tile([HS, 512], F32, tag="tt", name="rb")
                    nc.tensor.matmul(
                        rb, ones_bf[0:1, :], rcpb[0:1, e, :],
                        start=True, stop=True,
                    )
                    nc.vector.tensor_mul(
                        out=otnorm[:, e, :],
                        in0=otu_sb[0:HS, e, :],
                        in1=rb,
                    )
                    nc.sync.dma_start(
                        out=otn[64 * e:64 * e + HS, p,
                                m * 512:(m + 1) * 512],
                        in_=otnorm[:, e, :],
                    )

            def proj_tile(tt):
                y1 = ps_t.tile([128, 512], F32, tag="tt", name="y1")
                y2 = ps_t.tile([128, 256], F32, tag="tt", name="y2")
                for g in range(3):
                    lhs = otn[:, g, tt * 128:(tt + 1) * 128]
                    nc.tensor.matmul(
                        y1, lhs, wpt_sb[:, g, 0:512],
                        start=(g == 0), stop=(g == 2),
                    )
                    nc.tensor.matmul(
                        y2, lhs, wpt_sb[:, g, 512:768],
                        start=(g == 0), stop=(g == 2),
                    )
                ysb = ysbp.tile([128, C], F32, tag="ysb", name="ysb")
                nc.vector.tensor_copy(out=ysb[:, 0:512], in_=y1)
                nc.vector.tensor_copy(out=ysb[:, 512:768], in_=y2)
                eng = nc.sync if tt % 2 == 0 else nc.scalar
                eng.dma_start(out=y[tt * 128:(tt + 1) * 128, :], in_=ysb)

            def proj(m):
                for tt in range(4 * m, 4 * m + 4):
                    proj_tile(tt)

            # ---------------- pipelined program order --------------------
            # Units ordered so the ACT-heavy (m=3) exps sit mid-stream with
            # PE fillers (qk/v/proj) around them, and the LAST unit is the
            # smallest (0,2) so the tail is not exp-paced. Scores run one
            # unit ahead of pv (pts pool ring: max 32 tiles alive <= 36).
            def fill(fn, *args):
                return lambda: fn(*args)

            # filler placement rule: a qk_half/v_tile filler in block k is
            # consumed no earlier than block k+1 (or, within block k, at a
            # j position after its paced emission) — audited per block.
            qk_pair(0, 0)
            fused((0, 0), None,
                  [fill(qk_half, 0, 1, 0), fill(qk_half, 0, 1, 1),
                   fill(v_tile, 0), fill(v_tile, 1), fill(v_tile, 2),
                   fill(v_tile, 3)])
            fused((0, 1), (0, 0),
                  [fill(qk_half, 1, 0, 0), fill(qk_half, 1, 0, 1)])
            fused((1, 0), (0, 1),
                  [fill(qk_half, 1, 1, 0), fill(qk_half, 1, 1, 1),
                   fill(qk_half, 0, 2, 0), fill(v_tile, 4), fill(v_tile, 5)])
            fused((1, 1), (1, 0),
                  [fill(qk_half, 0, 2, 1), fill(v_tile, 6), fill(v_tile, 7),
                   fill(qk_half, 1, 2, 0), fill(qk_half, 1, 2, 1)])
            fused((1, 2), (1, 1),
                  [fill(qk_half, 2, 0, 0), fill(qk_half, 2, 0, 1)])
            fused((2, 0), (1, 2),
                  [fill(qk_half, 2, 1, 0), fill(qk_half, 2, 1, 1),
                   fill(v_tile, 8), fill(v_tile, 9)])
            fused((2, 1), (2, 0),
                  [fill(v_tile, 10), fill(v_tile, 11),
                   fill(qk_half, 2, 2, 0), fill(qk_half, 2, 2, 1)])
            fused((2, 2), (2, 1),
                  [fill(qk_half, 3, 0, 0), fill(qk_half, 3, 0, 1),
                   fill(proj_tile, 4), fill(proj_tile, 5)])
            fused((3, 0), (2, 2),
                  [fill(qk_half, 3, 1, 0), fill(qk_half, 3, 1, 1),
                   fill(v_tile, 12), fill(v_tile, 13),
                   fill(proj_tile, 6), fill(proj_tile, 7)])
            fused((3, 1), (3, 0),
                  [fill(v_tile, 14), fill(v_tile, 15),
                   fill(qk_half, 3, 2, 0), fill(qk_half, 3, 2, 1),
                   fill(proj_tile, 8), fill(proj_tile, 9)])
            fused((3, 2), (3, 1),
                  [fill(proj_tile, 10), fill(proj_tile, 11)])
            fused((0, 2), (3, 2))
            proj(3)
            pv_norm(0, 2)
            proj(0)

    nc.compile()
    return nc


_NC_CACHE = {}


def get_nc(repeat=1, phases=None):
    key = (repeat,)
    if key not in _NC_CACHE:
        nc = bacc.Bacc(
            "TRN2", target_bir_lowering=False, debug=False, num_devices=8
        )
        _NC_CACHE[key] = build_kernel(nc)
    return _NC_CACHE[key]


def make_in_maps(x, Wq, Wk, Wv, Wp):
    x = np.asarray(x, dtype=np.float32)
    Wq = np.asarray(Wq, dtype=np.float32)
    Wk = np.asarray(Wk, dtype=np.float32)
    Wv = np.asarray(Wv, dtype=np.float32)
    Wp = np.asarray(Wp, dtype=np.float32)
    bf = ml_dtypes.bfloat16
    in_maps = []
    for c in range(8):
        b = c // 2
        hs = HL * (c % 2)
        # x: [C, T] -> [128, NCT, T] (partition = row within 128-block)
        xh = np.ascontiguousarray(
            x[b].T.reshape(NCT, 128, T).transpose(1, 0, 2)
        ).astype(bf)
        # pair-stacked Q/K weights: [128, 3(pair), 2(qk), NCT, 128]
        w2 = np.empty((3, 2, NCT, 128, 128), dtype=np.float32)
        for qk, W in enumerate((Wq, Wk)):
            for p in range(3):
                pc = np.concatenate(
                    [W[hs + 2 * p], W[hs + 2 * p + 1]], axis=1
                )  # [C, 128]
                w2[p, qk] = pc.reshape(NCT, 128, 128)
        wqkp = np.ascontiguousarray(w2.transpose(3, 0, 1, 2, 4)).astype(bf)
        # wv: [128, NCT, HL*HS]
        wv_full = np.transpose(Wv[hs:hs + HL], (1, 0, 2)).reshape(C, HL * HS)
        wvh = np.ascontiguousarray(
            wv_full.reshape(NCT, 128, HL * HS).transpose(1, 0, 2)
        ).astype(bf)
        # wpt: Wp[:, i_slice].T -> [384, C] -> [128, 3, C]
        wpth = np.ascontiguousarray(
            Wp[:, hs * HS:(hs + HL) * HS].T.reshape(3, 128, C)
            .transpose(1, 0, 2)
        ).astype(bf)
        in_maps.append({"xh": xh, "wqkp": wqkp, "wvh": wvh, "wpth": wpth})
    return in_maps


def run(x, Wq, Wk, Wv, Wp, bp, trace=False):
    nc = get_nc()
    in_maps = make_in_maps(x, Wq, Wk, Wv, Wp)
    res = bass_utils.run_bass_kernel_spmd(
        nc, in_maps, core_ids=list(range(8)), trace=trace
    )
    y = np.zeros((B, T, C), dtype=np.float32)
    for c in range(8):
        y[c // 2] += res.results[c]["y"]
    y += np.asarray(bp, dtype=np.float32)
    return y, res


def kernel(x, Wq, Wk, Wv, Wp, bp):
    y, _ = run(x, Wq, Wk, Wv, Wp, bp)
    return y


def make_runner(nc):
    """Build the sharded PJRT callable once (mirrors the tail of
    bass2jax.run_bass_via_pjrt) so repeated timed executions don't re-trace.
    Returns (fn, prep) where prep(in_maps) device_puts the inputs and
    fn(device_inputs) -> per-core output dicts (blocking)."""
    import jax
    from jax.experimental.shard_map import shard_map
    from jax.sharding import Mesh, PartitionSpec, NamedSharding
    from concourse import mybir as _mybir
    from concourse.bass2jax import (
        _bass_exec_p, install_neuronx_cc_hook, partition_id_tensor,
    )

    install_neuronx_cc_hook()
    n_cores = 8
    partition_name = (
        nc.partition_id_tensor.name if nc.partition_id_tensor else None
    )
    in_names, out_names, out_avals = [], [], []
    for alloc in nc.m.functions[0].allocations:
        if not isinstance(alloc, _mybir.MemoryLocationSet):
            continue
        name = alloc.memorylocations[0].name
        if alloc.kind == "ExternalInput":
            if name != partition_name:
                in_names.append(name)
        elif alloc.kind == "ExternalOutput":
            out_names.append(name)
            out_avals.append(
                jax.core.ShapedArray(
                    tuple(alloc.tensor_shape), _mybir.dt.np(alloc.dtype)
                )
            )
    n_params = len(in_names)
    n_outs = len(out_avals)
    all_in_names = in_names + out_names
    if partition_name is not None:
        all_in_names.append(partition_name)

    def _body(*args):
        operands = list(args)
        if partition_name is not None:
            operands.append(partition_id_tensor())
        outs = _bass_exec_p.bind(
            *operands,
            out_avals=tuple(out_avals),
            in_names=tuple(all_in_names),
            out_names=tuple(out_names),
            lowering_input_output_aliases=(),
            sim_require_finite=True,
            sim_require_nnan=True,
            nc=nc,
        )
        return tuple(outs)

    devices = jax.devices()[:n_cores]
    mesh = Mesh(np.array(devices), ("core",))
    sharded = jax.jit(
        shard_map(
            _body, mesh=mesh,
            in_specs=(PartitionSpec("core"),) * (n_params + n_outs),
            out_specs=(PartitionSpec("core"),) * n_outs,
            check_rep=False,
        ),
        donate_argnums=tuple(range(n_params, n_params + n_outs)),
        keep_unused=True,
    )
    shd = NamedSharding(mesh, PartitionSpec("core"))

    def prep(in_maps):
        return [
            jax.device_put(
                np.concatenate([in_maps[c][nm] for c in range(n_cores)], axis=0),
                shd,
            )
            for nm in in_names
        ]

    def zeros():
        return [
            jax.device_put(
                np.zeros((n_cores * a.shape[0], *a.shape[1:]), a.dtype), shd
            )
            for a in out_avals
        ]

    def fn(dev_inputs, dev_zeros):
        outs = sharded(*dev_inputs, *dev_zeros)
        jax.block_until_ready(outs)
        return outs

    def make_loop_fn(n_iters):
        def _body_n(*args):
            ins = args[:n_params]
            carry = tuple(args[n_params:])

            def step(i, carry):
                operands = list(ins) + list(carry)
                if partition_name is not None:
                    operands.append(partition_id_tensor())
                outs = _bass_exec_p.bind(
                    *operands,
                    out_avals=tuple(out_avals),
                    in_names=tuple(all_in_names),
                    out_names=tuple(out_names),
                    lowering_input_output_aliases=(),
                    sim_require_finite=True,
                    sim_require_nnan=True,
                    nc=nc,
                )
                return tuple(outs)

            return jax.lax.fori_loop(0, n_iters, step, carry)

        looped = jax.jit(
            shard_map(
                _body_n, mesh=mesh,
                in_specs=(PartitionSpec("core"),) * (n_params + n_outs),
                out_specs=(PartitionSpec("core"),) * n_outs,
                check_rep=False,
            ),
            donate_argnums=tuple(range(n_params, n_params + n_outs)),
            keep_unused=True,
        )

        def run_n(dev_inputs, dev_zeros):
            outs = looped(*dev_inputs, *dev_zeros)
            jax.block_until_ready(outs)
            return outs

        return run_n

    return fn, prep, zeros, out_names, make_loop_fn


# revision 34
# speedup vs baseline: 1.1275x; 1.0122x over previous
"""Multi-head causal attention (B=4, T=2048, C=768, H=12, HS=64) on 8 trn2 cores.

Sharding: 48 (batch, head) units -> 6 per core. Core c: batch c//2, heads
6*(c%2) .. 6*(c%2)+6. Each core computes a partial output projection
y_partial[T, C] = sum over its 6 heads; host sums the two partials per batch
and adds the bias.

Per-core design (v2 — software-pipelined):
  All inputs are packed on the host in their final SBUF layout (leading 128
  partition dim) so each lands with ONE dma descriptor; x is loaded in four
  512-column chunks so compute starts after ~2 descriptors.

  Q/K weights are pair-stacked on the host (head 2p cols 0:64, head 2p+1
  cols 64:128), so the QK projection psum IS the pair-stacked pairQ/pairK
  layout and a single DVE copy per (p, qk, chunk) replaces the staging
  SBUF->SBUF DMAs.

  scores  ST[tk, tq] = matmul(lhsT=pairK 64-rows, rhs=pairQ 64-rows) with
          tile_position=(64e, 0): both heads of a pair run concurrently in
          disjoint PE row groups.
  softmax no max-subtraction (scores are O(+-8)); exp on ACT (bf16 out);
          row sums ride along as a ones-column appended to V (vaug col 64).
  P@V     OTu[65, tq] accumulated over tk tiles; row 64 = denominator.
  norm    reciprocal_approx_fast on the [1, 2, 512] denominator rows,
          bf16 K=1 broadcast matmul (fast; the fp32 one costs 1.1us/MM),
          one DVE multiply, DMA into pair-stacked otn.
  proj    y[tq, :] = sum_g matmul(lhsT=otn[:, g, tq], rhs=wpt[g]).

  Program order interleaves the 12 (m-chunk, pair) attention units with the
  projection chunks so the PE never idles >3us (HAM stays at K=8/8) and the
  ACT engine (exp: the co-bottleneck at ~120us) is fed continuously.
"""

import numpy as np
import ml_dtypes

import concourse.bacc as bacc
import concourse.bass as bass
import concourse.tile as tile
from concourse import mybir
from concourse import bass_utils

B, T, C = 4, 2048, 768
H, HS = 12, 64
HL = 6            # heads per core
NCT = C // 128    # 6 contraction tiles
NTT = T // 128    # 16 t tiles
NTC = T // 512    # 4 t chunks
SCALE = 1.0 / 8.0  # 1/sqrt(HS)

F32 = mybir.dt.float32
BF16 = mybir.dt.bfloat16


def build_kernel(nc):
    xh = nc.dram_tensor("xh", [128, NCT, T], BF16, kind="ExternalInput").ap()
    wqkp = nc.dram_tensor("wqkp", [128, 3, 2, NCT, 128], BF16,
                          kind="ExternalInput").ap()
    wvh = nc.dram_tensor("wvh", [128, NCT, HL * HS], BF16,
                         kind="ExternalInput").ap()
    wpth = nc.dram_tensor("wpth", [128, 3, C], BF16, kind="ExternalInput").ap()
    y = nc.dram_tensor("y", [T, C], F32, kind="ExternalOutput").ap()

    with tile.TileContext(nc) as tc:
        with (
            tc.tile_pool(name="consts", bufs=1) as consts,
            tc.tile_pool(name="xw", bufs=1) as xw,
            tc.tile_pool(name="pt", bufs=36) as ptp,
            tc.tile_pool(name="small", bufs=2) as small,
            tc.tile_pool(name="ysb", bufs=2) as ysbp,
            # PSUM: st 2x2 banks + otu 1x2 banks + tt 2x1 banks = 8 banks
            tc.tile_pool(name="ps_st", bufs=2, space="PSUM") as ps_st,
            tc.tile_pool(name="ps_otu", bufs=1, space="PSUM") as ps_otu,
            tc.tile_pool(name="ps_t", bufs=2, space="PSUM") as ps_t,
        ):
            # ------- input DMAs (one descriptor each, parallel queues) ----
            xt = xw.tile([128, NCT, T], BF16, tag="xt", name="xt")
            wqk_sb = xw.tile([128, 3, 2, NCT, 128], BF16, tag="wqk",
                             name="wqk")
            wv_sb = xw.tile([128, NCT, HL * HS], BF16, tag="wv", name="wv")
            wpt_sb = consts.tile([128, 3, C], BF16, tag="wpt", name="wpt")
            # per-pair weight DMAs so scores(0,0) starts as soon as pair 0
            # lands; x in column chunks; three parallel DMA queues
            nc.sync.dma_start(out=wqk_sb[:, 0], in_=wqkp[:, 0])
            nc.scalar.dma_start(out=xt[:, :, 0:512], in_=xh[:, :, 0:512])
            nc.gpsimd.dma_start(out=wv_sb, in_=wvh)
            nc.sync.dma_start(out=wqk_sb[:, 1], in_=wqkp[:, 1])
            nc.scalar.dma_start(out=xt[:, :, 512:1024], in_=xh[:, :, 512:1024])
            nc.sync.dma_start(out=wqk_sb[:, 2], in_=wqkp[:, 2])
            nc.gpsimd.dma_start(out=xt[:, :, 1024:1536],
                                in_=xh[:, :, 1024:1536])
            nc.gpsimd.dma_start(out=wpt_sb, in_=wpth)
            nc.gpsimd.dma_start(out=xt[:, :, 1536:2048],
                                in_=xh[:, :, 1536:2048])

            # warm the PE's HAM clock gate with dummy matmuls while the
            # input DMAs land: ~4-5us of continuous PE activity lifts the
            # clock from 1.2 to 2.4 GHz before the first real matmul
            warm = consts.tile([128, 64], BF16, tag="warm", name="warm")
            nc.gpsimd.memset(warm, 0.0)
            def warm_fill(n):
                # dummy matmuls to keep the PE's HAM activity monitor busy
                # across a known stall so the clock stays at 2.4 GHz
                wps = ps_t.tile([64, 64], F32, tag="tt", name="warmps")
                for _ in range(n):
                    nc.tensor.matmul(wps, warm, warm[:, 0:64],
                                     start=True, stop=True)

            warm_fill(120)

            # ---------------- persistent SBUF tensors --------------------
            vaug = consts.tile([128, NTT, HL, HS + 1], BF16)
            nc.gpsimd.memset(vaug[:, :, :, HS:HS + 1], 1.0)
            pairQ = consts.tile([128, 3, T], BF16, tag="pq", name="pq")
            pairK = consts.tile([128, 3, T], BF16, tag="pk", name="pk")
            otn = consts.tile([128, 3, T], BF16)
            ones_bf = consts.tile([1, HS], BF16)
            nc.gpsimd.memset(ones_bf, 1.0)

            # ---------------- phase subroutines --------------------------
            def qk_half(m, p, qk):
                sl = slice(m * 512, (m + 1) * 512)
                ps = ps_t.tile([128, 512], F32, tag="tt", name="psqk")
                for ci in range(NCT):
                    nc.tensor.matmul(
                        ps, wqk_sb[:, p, qk, ci, :], xt[:, ci, sl],
                        start=(ci == 0), stop=(ci == NCT - 1),
                    )
                dst = pairQ if qk == 0 else pairK
                nc.vector.tensor_copy(out=dst[:, p, sl], in_=ps)

            def qk_pair(m, p):
                qk_half(m, p, 0)
                qk_half(m, p, 1)

            def v_tile(tt):
                ps = ps_t.tile([128, HL * HS], F32, tag="tt", name="psv")
                for ci in range(NCT):
                    nc.tensor.matmul(
                        ps, xt[:, ci, tt * 128:(tt + 1) * 128],
                        wv_sb[:, ci, :],
                        start=(ci == 0), stop=(ci == NCT - 1),
                    )
                nc.vector.tensor_copy(
                    out=vaug[:, tt, :, 0:HS],
                    in_=ps.rearrange("p (h d) -> p h d", h=HL),
                )

            def v_chunk(m):
                for tt in range(4 * m, 4 * m + 4):
                    v_tile(tt)

            pts = {}  # (m, p) -> list of pt tiles

            def scores_j(m, p, j, unit):
                s0 = max(0, j - 4 * m)
                st = ps_st.tile([128, 2, 512], F32, tag="st", name="st")
                for e in range(2):
                    nc.tensor.matmul(
                        st[:, e, 128 * s0:512],
                        pairK[64 * e:64 * e + 64, p,
                              j * 128:(j + 1) * 128],
                        pairQ[64 * e:64 * e + 64, p,
                              m * 512 + 128 * s0:(m + 1) * 512],
                        start=True, stop=True,
                        tile_position=(64 * e, 0),
                    )
                pt = ptp.tile([128, 2, 512], BF16, tag="pt", name="pt")
                unit.append(pt)
                # one fused exp over both heads (2-bank strided AP)
                nc.scalar.activation(
                    out=pt[:, :, 128 * s0:512],
                    in_=st[:, :, 128 * s0:512],
                    func=mybir.ActivationFunctionType.Exp,
                    scale=SCALE,
                )
                if j >= 4 * m:
                    # zero the below-diagonal triangle of the diagonal
                    # subtile for both heads (keep where tq >= tk)
                    nc.gpsimd.affine_select(
                        out=pt[:, :, 128 * s0:128 * s0 + 128],
                        in_=pt[:, :, 128 * s0:128 * s0 + 128],
                        compare_op=mybir.AluOpType.is_ge,
                        fill=0.0, base=0,
                        pattern=[[0, 2], [1, 128]],
                        channel_multiplier=-1,
                    )

            def pv_j(m, p, j, unit, otu_ps):
                s0 = max(0, j - 4 * m)
                jmax = 4 * m + 3
                for e in range(2):
                    nc.tensor.matmul(
                        otu_ps[:, e, 128 * s0:512],
                        vaug[:, j, 2 * p + e, :],
                        unit[j][:, e, 128 * s0:512],
                        start=(j == 0), stop=(j == jmax),
                        skip_group_check=True,
                    )

            def scores(m, p):
                unit = []
                for j in range(4 * m + 4):
                    scores_j(m, p, j, unit)
                pts[(m, p)] = unit

            def fused(su, pu, fillers=()):
                """Interleave scores unit su's j-loop with pv unit pu's;
                `fillers` (closures of ~1us of PE work) are emitted at paced
                j positions so the PE stays busy while ACT drains the st
                ring (exp is slower per tile than the score matmuls)."""
                ms, ps_ = su
                s_unit = []
                if pu is not None:
                    mp, pp = pu
                    p_unit = pts.pop(pu)
                    otu_ps = ps_otu.tile([HS + 1, 2, 512], F32, tag="otu",
                                         name="otu")
                nj = 4 * ms + 4
                fillers = list(fillers)
                nf = len(fillers)
                fired = 0
                for j in range(max(nj, 4 * mp + 4 if pu is not None else 0)):
                    if j < nj:
                        scores_j(ms, ps_, j, s_unit)
                    if pu is not None and j < 4 * mp + 4:
                        pv_j(mp, pp, j, p_unit, otu_ps)
                    while fired < min(nf, nf * (j + 1) // max(nj, 1)):
                        fillers[fired]()
                        fired += 1
                for f in fillers[fired:]:
                    f()
                pts[su] = s_unit
                if pu is not None:
                    norm(pu, otu_ps)

            def pv_norm(m, p, fillers=()):
                unit = pts.pop((m, p))
                otu_ps = ps_otu.tile([HS + 1, 2, 512], F32, tag="otu",
                                     name="otu")
                nj = 4 * m + 4
                fillers = list(fillers)
                fired = 0
                for j in range(nj):
                    pv_j(m, p, j, unit, otu_ps)
                    while fired < len(fillers) * (j + 1) // nj:
                        fillers[fired]()
                        fired += 1
                norm((m, p), otu_ps)

            def norm(pu, otu_ps):
                # normalize: recip of the denominator rows, bf16 broadcast
                # matmul, one multiply per head
                m, p = pu
                # custom DVE ops ignore nonzero partition bases: copy the
                # denominator row to a partition-0 tile first (tensor_copy
                # does shift partitions; tensor_tensor cannot)
                den0 = small.tile([1, 2, 512], F32, tag="den0", name="den0")
                nc.vector.tensor_copy(out=den0, in_=otu_ps[HS:HS + 1, :, :])
                otu_sb = small.tile([HS + 1, 2, 512], F32, tag="otusb",
                                    name="otusb")
                nc.vector.tensor_copy(out=otu_sb, in_=otu_ps)
                rcp = small.tile([1, 2, 512], F32, tag="rcp", name="rcp")
                nc.vector.reciprocal_approx_fast(out=rcp, in_=den0)
                rcpb = small.tile([1, 2, 512], BF16, tag="rcpb", name="rcpb")
                nc.vector.tensor_copy(out=rcpb, in_=rcp)
                otnorm = small.tile([HS, 2, 512], BF16, tag="otnorm",
                                    name="otnorm")
                for e in range(2):
                    rb = ps_t.tile([HS, 512], F32, tag="tt", name="rb")
                    nc.tensor.matmul(
                        rb, ones_bf[0:1, :], rcpb[0:1, e, :],
                        start=True, stop=True,
                    )
                    nc.vector.tensor_mul(
                        out=otnorm[:, e, :],
                        in0=otu_sb[0:HS, e, :],
                        in1=rb,
                    )
                    nc.sync.dma_start(
                        out=otn[64 * e:64 * e + HS, p,
                                m * 512:(m + 1) * 512],
                        in_=otnorm[:, e, :],
                    )

            def proj_tile(tt):
                y1 = ps_t.tile([128, 512], F32, tag="tt", name="y1")
                y2 = ps_t.tile([128, 256], F32, tag="tt", name="y2")
                for g in range(3):
                    lhs = otn[:, g, tt * 128:(tt + 1) * 128]
                    nc.tensor.matmul(
                        y1, lhs, wpt_sb[:, g, 0:512],
                        start=(g == 0), stop=(g == 2),
                    )
                    nc.tensor.matmul(
                        y2, lhs, wpt_sb[:, g, 512:768],
                        start=(g == 0), stop=(g == 2),
                    )
                ysb = ysbp.tile([128, C], F32, tag="ysb", name="ysb")
                nc.vector.tensor_copy(out=ysb[:, 0:512], in_=y1)
                nc.vector.tensor_copy(out=ysb[:, 512:768], in_=y2)
                eng = nc.sync if tt % 2 == 0 else nc.scalar
                eng.dma_start(out=y[tt * 128:(tt + 1) * 128, :], in_=ysb)

            def proj(m):
                for tt in range(4 * m, 4 * m + 4):
                    proj_tile(tt)

            # ---------------- pipelined program order --------------------
            # Units ordered so the ACT-heavy (m=3) exps sit mid-stream with
            # PE fillers (qk/v/proj) around them, and the LAST unit is the
            # smallest (0,2) so the tail is not exp-paced. Scores run one
            # unit ahead of pv (pts pool ring: max 32 tiles alive <= 36).
            def fill(fn, *args):
                return lambda: fn(*args)

            # filler placement rule: a qk_half/v_tile filler in block k is
            # consumed no earlier than block k+1 (or, within block k, at a
            # j position after its paced emission) — audited per block.
            qk_pair(0, 0)
            fused((0, 0), None,
                  [fill(qk_half, 0, 1, 0), fill(qk_half, 0, 1, 1),
                   fill(v_tile, 0), fill(v_tile, 1), fill(v_tile, 2),
                   fill(v_tile, 3)])
            fused((0, 1), (0, 0),
                  [fill(qk_half, 1, 0, 0), fill(qk_half, 1, 0, 1)])
            fused((1, 0), (0, 1),
                  [fill(qk_half, 1, 1, 0), fill(qk_half, 1, 1, 1),
                   fill(qk_half, 0, 2, 0), fill(v_tile, 4), fill(v_tile, 5)])
            fused((1, 1), (1, 0),
                  [fill(qk_half, 0, 2, 1), fill(v_tile, 6), fill(v_tile, 7),
                   fill(qk_half, 1, 2, 0), fill(qk_half, 1, 2, 1)])
            fused((1, 2), (1, 1),
                  [fill(qk_half, 2, 0, 0), fill(qk_half, 2, 0, 1)])
            fused((2, 0), (1, 2),
                  [fill(qk_half, 2, 1, 0), fill(qk_half, 2, 1, 1),
                   fill(v_tile, 8), fill(v_tile, 9)])
            fused((2, 1), (2, 0),
                  [fill(v_tile, 10), fill(v_tile, 11),
                   fill(qk_half, 2, 2, 0), fill(qk_half, 2, 2, 1)])
            fused((2, 2), (2, 1),
                  [fill(qk_half, 3, 0, 0), fill(qk_half, 3, 0, 1),
                   fill(proj_tile, 4), fill(proj_tile, 5)])
            fused((3, 0), (2, 2),
                  [fill(qk_half, 3, 1, 0), fill(qk_half, 3, 1, 1),
                   fill(v_tile, 12), fill(v_tile, 13),
                   fill(proj_tile, 6), fill(proj_tile, 7)])
            fused((3, 1), (3, 0),
                  [fill(v_tile, 14), fill(v_tile, 15),
                   fill(qk_half, 3, 2, 0), fill(qk_half, 3, 2, 1),
                   fill(proj_tile, 8), fill(proj_tile, 9)])
            fused((0, 2), (3, 1),
                  [fill(proj_tile, 10), fill(proj_tile, 11)])
            fused((3, 2), (0, 2))
            proj_tile(0)
            proj_tile(1)
            pv_norm(3, 2, [fill(proj_tile, 2), fill(proj_tile, 3)])
            proj(3)

    nc.compile()
    return nc


_NC_CACHE = {}


def get_nc(repeat=1, phases=None):
    key = (repeat,)
    if key not in _NC_CACHE:
        nc = bacc.Bacc(
            "TRN2", target_bir_lowering=False, debug=False, num_devices=8
        )
        _NC_CACHE[key] = build_kernel(nc)
    return _NC_CACHE[key]


def make_in_maps(x, Wq, Wk, Wv, Wp):
    x = np.asarray(x, dtype=np.float32)
    Wq = np.asarray(Wq, dtype=np.float32)
    Wk = np.asarray(Wk, dtype=np.float32)
    Wv = np.asarray(Wv, dtype=np.float32)
    Wp = np.asarray(Wp, dtype=np.float32)
    bf = ml_dtypes.bfloat16
    in_maps = []
    for c in range(8):
        b = c // 2
        hs = HL * (c % 2)
        # x: [C, T] -> [128, NCT, T] (partition = row within 128-block)
        xh = np.ascontiguousarray(
            x[b].T.reshape(NCT, 128, T).transpose(1, 0, 2)
        ).astype(bf)
        # pair-stacked Q/K weights: [128, 3(pair), 2(qk), NCT, 128]
        w2 = np.empty((3, 2, NCT, 128, 128), dtype=np.float32)
        for qk, W in enumerate((Wq, Wk)):
            for p in range(3):
                pc = np.concatenate(
                    [W[hs + 2 * p], W[hs + 2 * p + 1]], axis=1
                )  # [C, 128]
                w2[p, qk] = pc.reshape(NCT, 128, 128)
        wqkp = np.ascontiguousarray(w2.transpose(3, 0, 1, 2, 4)).astype(bf)
        # wv: [128, NCT, HL*HS]
        wv_full = np.transpose(Wv[hs:hs + HL], (1, 0, 2)).reshape(C, HL * HS)
        wvh = np.ascontiguousarray(
            wv_full.reshape(NCT, 128, HL * HS).transpose(1, 0, 2)
        ).astype(bf)
        # wpt: Wp[:, i_slice].T -> [384, C] -> [128, 3, C]
        wpth = np.ascontiguousarray(
            Wp[:, hs * HS:(hs + HL) * HS].T.reshape(3, 128, C)
            .transpose(1, 0, 2)
        ).astype(bf)
        in_maps.append({"xh": xh, "wqkp": wqkp, "wvh": wvh, "wpth": wpth})
    return in_maps


def run(x, Wq, Wk, Wv, Wp, bp, trace=False):
    nc = get_nc()
    in_maps = make_in_maps(x, Wq, Wk, Wv, Wp)
    res = bass_utils.run_bass_kernel_spmd(
        nc, in_maps, core_ids=list(range(8)), trace=trace
    )
    y = np.zeros((B, T, C), dtype=np.float32)
    for c in range(8):
        y[c // 2] += res.results[c]["y"]
    y += np.asarray(bp, dtype=np.float32)
    return y, res


def kernel(x, Wq, Wk, Wv, Wp, bp):
    y, _ = run(x, Wq, Wk, Wv, Wp, bp)
    return y


def make_runner(nc):
    """Build the sharded PJRT callable once (mirrors the tail of
    bass2jax.run_bass_via_pjrt) so repeated timed executions don't re-trace.
    Returns (fn, prep) where prep(in_maps) device_puts the inputs and
    fn(device_inputs) -> per-core output dicts (blocking)."""
    import jax
    from jax.experimental.shard_map import shard_map
    from jax.sharding import Mesh, PartitionSpec, NamedSharding
    from concourse import mybir as _mybir
    from concourse.bass2jax import (
        _bass_exec_p, install_neuronx_cc_hook, partition_id_tensor,
    )

    install_neuronx_cc_hook()
    n_cores = 8
    partition_name = (
        nc.partition_id_tensor.name if nc.partition_id_tensor else None
    )
    in_names, out_names, out_avals = [], [], []
    for alloc in nc.m.functions[0].allocations:
        if not isinstance(alloc, _mybir.MemoryLocationSet):
            continue
        name = alloc.memorylocations[0].name
        if alloc.kind == "ExternalInput":
            if name != partition_name:
                in_names.append(name)
        elif alloc.kind == "ExternalOutput":
            out_names.append(name)
            out_avals.append(
                jax.core.ShapedArray(
                    tuple(alloc.tensor_shape), _mybir.dt.np(alloc.dtype)
                )
            )
    n_params = len(in_names)
    n_outs = len(out_avals)
    all_in_names = in_names + out_names
    if partition_name is not None:
        all_in_names.append(partition_name)

    def _body(*args):
        operands = list(args)
        if partition_name is not None:
            operands.append(partition_id_tensor())
        outs = _bass_exec_p.bind(
            *operands,
            out_avals=tuple(out_avals),
            in_names=tuple(all_in_names),
            out_names=tuple(out_names),
            lowering_input_output_aliases=(),
            sim_require_finite=True,
            sim_require_nnan=True,
            nc=nc,
        )
        return tuple(outs)

    devices = jax.devices()[:n_cores]
    mesh = Mesh(np.array(devices), ("core",))
    sharded = jax.jit(
        shard_map(
            _body, mesh=mesh,
            in_specs=(PartitionSpec("core"),) * (n_params + n_outs),
            out_specs=(PartitionSpec("core"),) * n_outs,
            check_rep=False,
        ),
        donate_argnums=tuple(range(n_params, n_params + n_outs)),
        keep_unused=True,
    )
    shd = NamedSharding(mesh, PartitionSpec("core"))

    def prep(in_maps):
        return [
            jax.device_put(
                np.concatenate([in_maps[c][nm] for c in range(n_cores)], axis=0),
                shd,
            )
            for nm in in_names
        ]

    def zeros():
        return [
            jax.device_put(
                np.zeros((n_cores * a.shape[0], *a.shape[1:]), a.dtype), shd
            )
            for a in out_avals
        ]

    def fn(dev_inputs, dev_zeros):
        outs = sharded(*dev_inputs, *dev_zeros)
        jax.block_until_ready(outs)
        return outs

    def make_loop_fn(n_iters):
        def _body_n(*args):
            ins = args[:n_params]
            carry = tuple(args[n_params:])

            def step(i, carry):
                operands = list(ins) + list(carry)
                if partition_name is not None:
                    operands.append(partition_id_tensor())
                outs = _bass_exec_p.bind(
                    *operands,
                    out_avals=tuple(out_avals),
                    in_names=tuple(all_in_names),
                    out_names=tuple(out_names),
                    lowering_input_output_aliases=(),
                    sim_require_finite=True,
                    sim_require_nnan=True,
                    nc=nc,
                )
                return tuple(outs)

            return jax.lax.fori_loop(0, n_iters, step, carry)

        looped = jax.jit(
            shard_map(
                _body_n, mesh=mesh,
                in_specs=(PartitionSpec("core"),) * (n_params + n_outs),
                out_specs=(PartitionSpec("core"),) * n_outs,
                check_rep=False,
            ),
            donate_argnums=tuple(range(n_params, n_params + n_outs)),
            keep_unused=True,
        )

        def run_n(dev_inputs, dev_zeros):
            outs = looped(*dev_inputs, *dev_zeros)
            jax.block_until_ready(outs)
            return outs

        return run_n

    return fn, prep, zeros, out_names, make_loop_fn


# revision 36
# speedup vs baseline: 1.1453x; 1.0158x over previous
"""Multi-head causal attention (B=4, T=2048, C=768, H=12, HS=64) on 8 trn2 cores.

Sharding: 48 (batch, head) units -> 6 per core. Core c: batch c//2, heads
6*(c%2) .. 6*(c%2)+6. Each core computes a partial output projection
y_partial[T, C] = sum over its 6 heads; host sums the two partials per batch
and adds the bias.

Per-core design (v2 — software-pipelined):
  All inputs are packed on the host in their final SBUF layout (leading 128
  partition dim) so each lands with ONE dma descriptor; x is loaded in four
  512-column chunks so compute starts after ~2 descriptors.

  Q/K weights are pair-stacked on the host (head 2p cols 0:64, head 2p+1
  cols 64:128), so the QK projection psum IS the pair-stacked pairQ/pairK
  layout and a single DVE copy per (p, qk, chunk) replaces the staging
  SBUF->SBUF DMAs.

  scores  ST[tk, tq] = matmul(lhsT=pairK 64-rows, rhs=pairQ 64-rows) with
          tile_position=(64e, 0): both heads of a pair run concurrently in
          disjoint PE row groups.
  softmax no max-subtraction (scores are O(+-8)); exp on ACT (bf16 out);
          row sums ride along as a ones-column appended to V (vaug col 64).
  P@V     OTu[65, tq] accumulated over tk tiles; row 64 = denominator.
  norm    reciprocal_approx_fast on the [1, 2, 512] denominator rows,
          bf16 K=1 broadcast matmul (fast; the fp32 one costs 1.1us/MM),
          one DVE multiply, DMA into pair-stacked otn.
  proj    y[tq, :] = sum_g matmul(lhsT=otn[:, g, tq], rhs=wpt[g]).

  Program order interleaves the 12 (m-chunk, pair) attention units with the
  projection chunks so the PE never idles >3us (HAM stays at K=8/8) and the
  ACT engine (exp: the co-bottleneck at ~120us) is fed continuously.
"""

import numpy as np
import ml_dtypes

import concourse.bacc as bacc
import concourse.bass as bass
import concourse.tile as tile
from concourse import mybir
from concourse import bass_utils

B, T, C = 4, 2048, 768
H, HS = 12, 64
HL = 6            # heads per core
NCT = C // 128    # 6 contraction tiles
NTT = T // 128    # 16 t tiles
NTC = T // 512    # 4 t chunks
SCALE = 1.0 / 8.0  # 1/sqrt(HS)

F32 = mybir.dt.float32
BF16 = mybir.dt.bfloat16


def build_kernel(nc):
    xh = nc.dram_tensor("xh", [128, NCT, T], BF16, kind="ExternalInput").ap()
    wqkp = nc.dram_tensor("wqkp", [128, 3, 2, NCT, 128], BF16,
                          kind="ExternalInput").ap()
    wvh = nc.dram_tensor("wvh", [128, NCT, HL * HS], BF16,
                         kind="ExternalInput").ap()
    wpth = nc.dram_tensor("wpth", [128, 3, C], BF16, kind="ExternalInput").ap()
    y = nc.dram_tensor("y", [T, C], F32, kind="ExternalOutput").ap()

    with tile.TileContext(nc) as tc:
        with (
            tc.tile_pool(name="consts", bufs=1) as consts,
            tc.tile_pool(name="xw", bufs=1) as xw,
            tc.tile_pool(name="pt", bufs=36) as ptp,
            tc.tile_pool(name="small", bufs=2) as small,
            tc.tile_pool(name="ysb", bufs=2) as ysbp,
            # PSUM: st 2x2 banks + otu 1x2 banks + tt 2x1 banks = 8 banks
            tc.tile_pool(name="ps_st", bufs=2, space="PSUM") as ps_st,
            tc.tile_pool(name="ps_otu", bufs=1, space="PSUM") as ps_otu,
            tc.tile_pool(name="ps_t", bufs=2, space="PSUM") as ps_t,
        ):
            # ------- input DMAs (one descriptor each, parallel queues) ----
            xt = xw.tile([128, NCT, T], BF16, tag="xt", name="xt")
            wqk_sb = xw.tile([128, 3, 2, NCT, 128], BF16, tag="wqk",
                             name="wqk")
            wv_sb = xw.tile([128, NCT, HL * HS], BF16, tag="wv", name="wv")
            wpt_sb = consts.tile([128, 3, C], BF16, tag="wpt", name="wpt")
            # per-pair weight DMAs so scores(0,0) starts as soon as pair 0
            # lands; x in column chunks; three parallel DMA queues
            nc.sync.dma_start(out=wqk_sb[:, 0], in_=wqkp[:, 0])
            nc.scalar.dma_start(out=xt[:, :, 0:512], in_=xh[:, :, 0:512])
            nc.gpsimd.dma_start(out=wv_sb, in_=wvh)
            nc.sync.dma_start(out=wqk_sb[:, 1], in_=wqkp[:, 1])
            nc.scalar.dma_start(out=xt[:, :, 512:1024], in_=xh[:, :, 512:1024])
            nc.sync.dma_start(out=wqk_sb[:, 2], in_=wqkp[:, 2])
            nc.gpsimd.dma_start(out=xt[:, :, 1024:1536],
                                in_=xh[:, :, 1024:1536])
            nc.gpsimd.dma_start(out=wpt_sb, in_=wpth)
            nc.gpsimd.dma_start(out=xt[:, :, 1536:2048],
                                in_=xh[:, :, 1536:2048])

            # warm the PE's HAM clock gate with dummy matmuls while the
            # input DMAs land: ~4-5us of continuous PE activity lifts the
            # clock from 1.2 to 2.4 GHz before the first real matmul
            warm = consts.tile([128, 64], BF16, tag="warm", name="warm")
            nc.gpsimd.memset(warm, 0.0)
            def warm_fill(n):
                # dummy matmuls to keep the PE's HAM activity monitor busy
                # across a known stall so the clock stays at 2.4 GHz
                wps = ps_t.tile([64, 64], F32, tag="tt", name="warmps")
                for _ in range(n):
                    nc.tensor.matmul(wps, warm, warm[:, 0:64],
                                     start=True, stop=True)

            warm_fill(150)

            # ---------------- persistent SBUF tensors --------------------
            vaug = consts.tile([128, NTT, HL, HS + 1], BF16)
            nc.gpsimd.memset(vaug[:, :, :, HS:HS + 1], 1.0)
            pairQ = consts.tile([128, 3, T], BF16, tag="pq", name="pq")
            pairK = consts.tile([128, 3, T], BF16, tag="pk", name="pk")
            otn = consts.tile([128, 3, T], BF16)
            ones_bf = consts.tile([1, HS], BF16)
            nc.gpsimd.memset(ones_bf, 1.0)

            # ---------------- phase subroutines --------------------------
            def qk_half(m, p, qk):
                sl = slice(m * 512, (m + 1) * 512)
                ps = ps_t.tile([128, 512], F32, tag="tt", name="psqk")
                for ci in range(NCT):
                    nc.tensor.matmul(
                        ps, wqk_sb[:, p, qk, ci, :], xt[:, ci, sl],
                        start=(ci == 0), stop=(ci == NCT - 1),
                    )
                dst = pairQ if qk == 0 else pairK
                nc.vector.tensor_copy(out=dst[:, p, sl], in_=ps)

            def qk_pair(m, p):
                qk_half(m, p, 0)
                qk_half(m, p, 1)

            def v_tile(tt):
                ps = ps_t.tile([128, HL * HS], F32, tag="tt", name="psv")
                for ci in range(NCT):
                    nc.tensor.matmul(
                        ps, xt[:, ci, tt * 128:(tt + 1) * 128],
                        wv_sb[:, ci, :],
                        start=(ci == 0), stop=(ci == NCT - 1),
                    )
                nc.vector.tensor_copy(
                    out=vaug[:, tt, :, 0:HS],
                    in_=ps.rearrange("p (h d) -> p h d", h=HL),
                )

            def v_chunk(m):
                for tt in range(4 * m, 4 * m + 4):
                    v_tile(tt)

            pts = {}  # (m, p) -> list of pt tiles

            def scores_j(m, p, j, unit):
                s0 = max(0, j - 4 * m)
                st = ps_st.tile([128, 2, 512], F32, tag="st", name="st")
                for e in range(2):
                    nc.tensor.matmul(
                        st[:, e, 128 * s0:512],
                        pairK[64 * e:64 * e + 64, p,
                              j * 128:(j + 1) * 128],
                        pairQ[64 * e:64 * e + 64, p,
                              m * 512 + 128 * s0:(m + 1) * 512],
                        start=True, stop=True,
                        tile_position=(64 * e, 0),
                    )
                pt = ptp.tile([128, 2, 512], BF16, tag="pt", name="pt")
                unit.append(pt)
                # one fused exp over both heads (2-bank strided AP)
                nc.scalar.activation(
                    out=pt[:, :, 128 * s0:512],
                    in_=st[:, :, 128 * s0:512],
                    func=mybir.ActivationFunctionType.Exp,
                    scale=SCALE,
                )
                if j >= 4 * m:
                    # zero the below-diagonal triangle of the diagonal
                    # subtile for both heads (keep where tq >= tk)
                    nc.gpsimd.affine_select(
                        out=pt[:, :, 128 * s0:128 * s0 + 128],
                        in_=pt[:, :, 128 * s0:128 * s0 + 128],
                        compare_op=mybir.AluOpType.is_ge,
                        fill=0.0, base=0,
                        pattern=[[0, 2], [1, 128]],
                        channel_multiplier=-1,
                    )

            def pv_j(m, p, j, unit, otu_ps):
                s0 = max(0, j - 4 * m)
                jmax = 4 * m + 3
                for e in range(2):
                    nc.tensor.matmul(
                        otu_ps[:, e, 128 * s0:512],
                        vaug[:, j, 2 * p + e, :],
                        unit[j][:, e, 128 * s0:512],
                        start=(j == 0), stop=(j == jmax),
                        skip_group_check=True,
                    )

            def scores(m, p):
                unit = []
                for j in range(4 * m + 4):
                    scores_j(m, p, j, unit)
                pts[(m, p)] = unit

            def fused(su, pu, fillers=()):
                """Interleave scores unit su's j-loop with pv unit pu's;
                `fillers` (closures of ~1us of PE work) are emitted at paced
                j positions so the PE stays busy while ACT drains the st
                ring (exp is slower per tile than the score matmuls)."""
                ms, ps_ = su
                s_unit = []
                if pu is not None:
                    mp, pp = pu
                    p_unit = pts.pop(pu)
                    otu_ps = ps_otu.tile([HS + 1, 2, 512], F32, tag="otu",
                                         name="otu")
                nj = 4 * ms + 4
                fillers = list(fillers)
                nf = len(fillers)
                fired = 0
                for j in range(max(nj, 4 * mp + 4 if pu is not None else 0)):
                    if j < nj:
                        scores_j(ms, ps_, j, s_unit)
                    if pu is not None and j < 4 * mp + 4:
                        pv_j(mp, pp, j, p_unit, otu_ps)
                    while fired < min(nf, nf * (j + 1) // max(nj, 1)):
                        fillers[fired]()
                        fired += 1
                for f in fillers[fired:]:
                    f()
                pts[su] = s_unit
                if pu is not None:
                    norm(pu, otu_ps)

            def pv_norm(m, p, fillers=()):
                unit = pts.pop((m, p))
                otu_ps = ps_otu.tile([HS + 1, 2, 512], F32, tag="otu",
                                     name="otu")
                nj = 4 * m + 4
                fillers = list(fillers)
                fired = 0
                for j in range(nj):
                    pv_j(m, p, j, unit, otu_ps)
                    while fired < len(fillers) * (j + 1) // nj:
                        fillers[fired]()
                        fired += 1
                norm((m, p), otu_ps)

            def norm(pu, otu_ps):
                # normalize: recip of the denominator rows, bf16 broadcast
                # matmul, one multiply per head
                m, p = pu
                # custom DVE ops ignore nonzero partition bases: copy the
                # denominator row to a partition-0 tile first (tensor_copy
                # does shift partitions; tensor_tensor cannot)
                den0 = small.tile([1, 2, 512], F32, tag="den0", name="den0")
                nc.vector.tensor_copy(out=den0, in_=otu_ps[HS:HS + 1, :, :])
                otu_sb = small.tile([HS + 1, 2, 512], F32, tag="otusb",
                                    name="otusb")
                nc.vector.tensor_copy(out=otu_sb, in_=otu_ps)
                rcp = small.tile([1, 2, 512], F32, tag="rcp", name="rcp")
                nc.vector.reciprocal_approx_fast(out=rcp, in_=den0)
                rcpb = small.tile([1, 2, 512], BF16, tag="rcpb", name="rcpb")
                nc.vector.tensor_copy(out=rcpb, in_=rcp)
                otnorm = small.tile([HS, 2, 512], BF16, tag="otnorm",
                                    name="otnorm")
                for e in range(2):
                    rb = ps_t.tile([HS, 512], F32, tag="tt", name="rb")
                    nc.tensor.matmul(
                        rb, ones_bf[0:1, :], rcpb[0:1, e, :],
                        start=True, stop=True,
                    )
                    nc.vector.tensor_mul(
                        out=otnorm[:, e, :],
                        in0=otu_sb[0:HS, e, :],
                        in1=rb,
                    )
                    nc.sync.dma_start(
                        out=otn[64 * e:64 * e + HS, p,
                                m * 512:(m + 1) * 512],
                        in_=otnorm[:, e, :],
                    )

            def proj_tile(tt):
                y1 = ps_t.tile([128, 512], F32, tag="tt", name="y1")
                y2 = ps_t.tile([128, 256], F32, tag="tt", name="y2")
                for g in range(3):
                    lhs = otn[:, g, tt * 128:(tt + 1) * 128]
                    nc.tensor.matmul(
                        y1, lhs, wpt_sb[:, g, 0:512],
                        start=(g == 0), stop=(g == 2),
                    )
                    nc.tensor.matmul(
                        y2, lhs, wpt_sb[:, g, 512:768],
                        start=(g == 0), stop=(g == 2),
                    )
                ysb = ysbp.tile([128, C], F32, tag="ysb", name="ysb")
                nc.vector.tensor_copy(out=ysb[:, 0:512], in_=y1)
                nc.vector.tensor_copy(out=ysb[:, 512:768], in_=y2)
                eng = nc.sync if tt % 2 == 0 else nc.scalar
                eng.dma_start(out=y[tt * 128:(tt + 1) * 128, :], in_=ysb)

            def proj(m):
                for tt in range(4 * m, 4 * m + 4):
                    proj_tile(tt)

            # ---------------- pipelined program order --------------------
            # Units ordered so the ACT-heavy (m=3) exps sit mid-stream with
            # PE fillers (qk/v/proj) around them, and the LAST unit is the
            # smallest (0,2) so the tail is not exp-paced. Scores run one
            # unit ahead of pv (pts pool ring: max 32 tiles alive <= 36).
            def fill(fn, *args):
                return lambda: fn(*args)

            # filler placement rule: a qk_half/v_tile filler in block k is
            # consumed no earlier than block k+1 (or, within block k, at a
            # j position after its paced emission) — audited per block.
            qk_pair(0, 0)
            fused((0, 0), None,
                  [fill(qk_half, 0, 1, 0), fill(qk_half, 0, 1, 1),
                   fill(v_tile, 0), fill(v_tile, 1), fill(v_tile, 2),
                   fill(v_tile, 3)])
            fused((0, 1), (0, 0),
                  [fill(qk_half, 1, 0, 0), fill(qk_half, 1, 0, 1)])
            fused((1, 0), (0, 1),
                  [fill(qk_half, 1, 1, 0), fill(qk_half, 1, 1, 1),
                   fill(qk_half, 0, 2, 0), fill(v_tile, 4), fill(v_tile, 5)])
            fused((1, 1), (1, 0),
                  [fill(qk_half, 0, 2, 1), fill(v_tile, 6), fill(v_tile, 7),
                   fill(qk_half, 1, 2, 0), fill(qk_half, 1, 2, 1)])
            fused((1, 2), (1, 1),
                  [fill(qk_half, 2, 0, 0), fill(qk_half, 2, 0, 1)])
            fused((2, 0), (1, 2),
                  [fill(qk_half, 2, 1, 0), fill(qk_half, 2, 1, 1),
                   fill(v_tile, 8), fill(v_tile, 9)])
            fused((2, 1), (2, 0),
                  [fill(v_tile, 10), fill(v_tile, 11),
                   fill(qk_half, 2, 2, 0), fill(qk_half, 2, 2, 1)])
            fused((2, 2), (2, 1),
                  [fill(qk_half, 3, 0, 0), fill(qk_half, 3, 0, 1),
                   fill(proj_tile, 4), fill(proj_tile, 5)])
            fused((3, 0), (2, 2),
                  [fill(qk_half, 3, 1, 0), fill(qk_half, 3, 1, 1),
                   fill(v_tile, 12), fill(v_tile, 13),
                   fill(proj_tile, 6), fill(proj_tile, 7)])
            fused((3, 1), (3, 0),
                  [fill(v_tile, 14), fill(v_tile, 15),
                   fill(qk_half, 3, 2, 0), fill(qk_half, 3, 2, 1),
                   fill(proj_tile, 8), fill(proj_tile, 9)])
            fused((0, 2), (3, 1),
                  [fill(proj_tile, 10), fill(proj_tile, 11)])
            fused((3, 2), (0, 2))
            proj_tile(0)
            proj_tile(1)
            pv_norm(3, 2, [fill(proj_tile, 2), fill(warm_fill, 14),
                           fill(proj_tile, 3), fill(warm_fill, 14),
                           fill(warm_fill, 14)])
            warm_fill(40)
            proj(3)

    nc.compile()
    return nc


_NC_CACHE = {}


def get_nc(repeat=1, phases=None):
    key = (repeat,)
    if key not in _NC_CACHE:
        nc = bacc.Bacc(
            "TRN2", target_bir_lowering=False, debug=False, num_devices=8
        )
        _NC_CACHE[key] = build_kernel(nc)
    return _NC_CACHE[key]


def make_in_maps(x, Wq, Wk, Wv, Wp):
    x = np.asarray(x, dtype=np.float32)
    Wq = np.asarray(Wq, dtype=np.float32)
    Wk = np.asarray(Wk, dtype=np.float32)
    Wv = np.asarray(Wv, dtype=np.float32)
    Wp = np.asarray(Wp, dtype=np.float32)
    bf = ml_dtypes.bfloat16
    in_maps = []
    for c in range(8):
        b = c // 2
        hs = HL * (c % 2)
        # x: [C, T] -> [128, NCT, T] (partition = row within 128-block)
        xh = np.ascontiguousarray(
            x[b].T.reshape(NCT, 128, T).transpose(1, 0, 2)
        ).astype(bf)
        # pair-stacked Q/K weights: [128, 3(pair), 2(qk), NCT, 128]
        w2 = np.empty((3, 2, NCT, 128, 128), dtype=np.float32)
        for qk, W in enumerate((Wq, Wk)):
            for p in range(3):
                pc = np.concatenate(
                    [W[hs + 2 * p], W[hs + 2 * p + 1]], axis=1
                )  # [C, 128]
                w2[p, qk] = pc.reshape(NCT, 128, 128)
        wqkp = np.ascontiguousarray(w2.transpose(3, 0, 1, 2, 4)).astype(bf)
        # wv: [128, NCT, HL*HS]
        wv_full = np.transpose(Wv[hs:hs + HL], (1, 0, 2)).reshape(C, HL * HS)
        wvh = np.ascontiguousarray(
            wv_full.reshape(NCT, 128, HL * HS).transpose(1, 0, 2)
        ).astype(bf)
        # wpt: Wp[:, i_slice].T -> [384, C] -> [128, 3, C]
        wpth = np.ascontiguousarray(
            Wp[:, hs * HS:(hs + HL) * HS].T.reshape(3, 128, C)
            .transpose(1, 0, 2)
        ).astype(bf)
        in_maps.append({"xh": xh, "wqkp": wqkp, "wvh": wvh, "wpth": wpth})
    return in_maps


def run(x, Wq, Wk, Wv, Wp, bp, trace=False):
    nc = get_nc()
    in_maps = make_in_maps(x, Wq, Wk, Wv, Wp)
    res = bass_utils.run_bass_kernel_spmd(
        nc, in_maps, core_ids=list(range(8)), trace=trace
    )
    y = np.zeros((B, T, C), dtype=np.float32)
    for c in range(8):
        y[c // 2] += res.results[c]["y"]
    y += np.asarray(bp, dtype=np.float32)
    return y, res


def kernel(x, Wq, Wk, Wv, Wp, bp):
    y, _ = run(x, Wq, Wk, Wv, Wp, bp)
    return y


def make_runner(nc):
    """Build the sharded PJRT callable once (mirrors the tail of
    bass2jax.run_bass_via_pjrt) so repeated timed executions don't re-trace.
    Returns (fn, prep) where prep(in_maps) device_puts the inputs and
    fn(device_inputs) -> per-core output dicts (blocking)."""
    import jax
    from jax.experimental.shard_map import shard_map
    from jax.sharding import Mesh, PartitionSpec, NamedSharding
    from concourse import mybir as _mybir
    from concourse.bass2jax import (
        _bass_exec_p, install_neuronx_cc_hook, partition_id_tensor,
    )

    install_neuronx_cc_hook()
    n_cores = 8
    partition_name = (
        nc.partition_id_tensor.name if nc.partition_id_tensor else None
    )
    in_names, out_names, out_avals = [], [], []
    for alloc in nc.m.functions[0].allocations:
        if not isinstance(alloc, _mybir.MemoryLocationSet):
            continue
        name = alloc.memorylocations[0].name
        if alloc.kind == "ExternalInput":
            if name != partition_name:
                in_names.append(name)
        elif alloc.kind == "ExternalOutput":
            out_names.append(name)
            out_avals.append(
                jax.core.ShapedArray(
                    tuple(alloc.tensor_shape), _mybir.dt.np(alloc.dtype)
                )
            )
    n_params = len(in_names)
    n_outs = len(out_avals)
    all_in_names = in_names + out_names
    if partition_name is not None:
        all_in_names.append(partition_name)

    def _body(*args):
        operands = list(args)
        if partition_name is not None:
            operands.append(partition_id_tensor())
        outs = _bass_exec_p.bind(
            *operands,
            out_avals=tuple(out_avals),
            in_names=tuple(all_in_names),
            out_names=tuple(out_names),
            lowering_input_output_aliases=(),
            sim_require_finite=True,
            sim_require_nnan=True,
            nc=nc,
        )
        return tuple(outs)

    devices = jax.devices()[:n_cores]
    mesh = Mesh(np.array(devices), ("core",))
    sharded = jax.jit(
        shard_map(
            _body, mesh=mesh,
            in_specs=(PartitionSpec("core"),) * (n_params + n_outs),
            out_specs=(PartitionSpec("core"),) * n_outs,
            check_rep=False,
        ),
        donate_argnums=tuple(range(n_params, n_params + n_outs)),
        keep_unused=True,
    )
    shd = NamedSharding(mesh, PartitionSpec("core"))

    def prep(in_maps):
        return [
            jax.device_put(
                np.concatenate([in_maps[c][nm] for c in range(n_cores)], axis=0),
                shd,
            )
            for nm in in_names
        ]

    def zeros():
        return [
            jax.device_put(
                np.zeros((n_cores * a.shape[0], *a.shape[1:]), a.dtype), shd
            )
            for a in out_avals
        ]

    def fn(dev_inputs, dev_zeros):
        outs = sharded(*dev_inputs, *dev_zeros)
        jax.block_until_ready(outs)
        return outs

    def make_loop_fn(n_iters):
        def _body_n(*args):
            ins = args[:n_params]
            carry = tuple(args[n_params:])

            def step(i, carry):
                operands = list(ins) + list(carry)
                if partition_name is not None:
                    operands.append(partition_id_tensor())
                outs = _bass_exec_p.bind(
                    *operands,
                    out_avals=tuple(out_avals),
                    in_names=tuple(all_in_names),
                    out_names=tuple(out_names),
                    lowering_input_output_aliases=(),
                    sim_require_finite=True,
                    sim_require_nnan=True,
                    nc=nc,
                )
                return tuple(outs)

            return jax.lax.fori_loop(0, n_iters, step, carry)

        looped = jax.jit(
            shard_map(
                _body_n, mesh=mesh,
                in_specs=(PartitionSpec("core"),) * (n_params + n_outs),
                out_specs=(PartitionSpec("core"),) * n_outs,
                check_rep=False,
            ),
            donate_argnums=tuple(range(n_params, n_params + n_outs)),
            keep_unused=True,
        )

        def run_n(dev_inputs, dev_zeros):
            outs = looped(*dev_inputs, *dev_zeros)
            jax.block_until_ready(outs)
            return outs

        return run_n

    return fn, prep, zeros, out_names, make_loop_fn
